# revision 6
# baseline (speedup 1.0000x reference)
"""CapsuleLayer dynamic-routing kernel for 8 Trainium2 NeuronCores.

Data-parallel over batch (32 per core), W replicated. Per core:
  hat = einsum('bie,ijed->bijd') kept in SBUF f16, layout
  [p=(i%16)*8+(b%8), free=(c=i//16, g=b//8, d, j)].
  hat built by PE: stationary = host-built block-diag x matrices
  (ablk), moving = W chunks; s0 for routing iter 0 comes directly from
  inpT x W matmuls (uniform coupling).
Routing (3 iters, 2 fused passes):
  agreement  a=<hat,v>: DVE f16 mult + d-halving tree (2x mode).
  softmax    ACT exp + DVE reduce/recip.
  s = sum_i c*hat: per-(c,g,j) PE matmuls with c-selector stationaries
  (Csel[k=(i,b8), m=b8'] = c*delta), accumulated in PSUM -> no DVE mult.
"""

import sys
from contextlib import ExitStack

import numpy as np

sys.path.insert(0, "/opt/trn_rl_repo")

import ml_dtypes  # noqa: E402

F16 = ml_dtypes.float16 if hasattr(ml_dtypes, "float16") else np.float16

B, I, E = 256, 1152, 8
J, D = 10, 16
NCORES = 8
BL = B // NCORES          # 32 batches per core
C = I // 16               # 72 i-chunks of 16
G = BL // 8               # 4 b-groups of 8
JD = J * D                # 160
GJD = G * JD              # 640
CGJ = C * G * J           # 2880
FREE = C * G * JD         # 46080 free elems of hat per partition
SLAB = 9                  # c-chunks per slab
NSLAB = C // SLAB         # 8
SF = SLAB * GJD           # 5760 hat elems per slab per partition
SN = SLAB * G * J         # 360 (c,g,j) nodes per slab
NR = 3

# evac tiles handled by DVE (fills pre-v0 idle window); rest go to ACT
DVE_EVACS = 24


def _build_kernel():
    import concourse.bass as bass
    import concourse.bacc as bacc
    import concourse.tile as tile
    from concourse import mybir

    fp32 = mybir.dt.float32
    f16 = mybir.dt.float16
    ADD = mybir.AluOpType.add
    MUL = mybir.AluOpType.mult

    nc = bacc.Bacc("TRN2")
    t_wa = nc.dram_tensor("wa", [128, C * JD], f16, kind="ExternalInput")
    t_inpT = nc.dram_tensor("inpT", [128, C * BL], f16, kind="ExternalInput")
    t_ablk = nc.dram_tensor("ablk", [128, C * G * 128], f16,
                            kind="ExternalInput")
    t_biasl = nc.dram_tensor("biasl", [128, C * J], f16, kind="ExternalInput")
    t_m8x = nc.dram_tensor("m8x", [128, 8 * SN], f16, kind="ExternalInput")
    t_s8 = nc.dram_tensor("s8", [8, 128], f16, kind="ExternalInput")
    t_s32 = nc.dram_tensor("s32", [32, 512], f16, kind="ExternalInput")
    t_out = nc.dram_tensor("out", [8, GJD], fp32, kind="ExternalOutput")

    def bcast(ap, pos, n):
        """Insert a broadcast (step 0, count n) free dim at free-pos pos."""
        lst = [list(x) for x in ap.ap]
        lst.insert(1 + pos, [0, n])
        return bass.AP(tensor=ap.tensor, offset=ap.offset, ap=lst)

    with ExitStack() as ctx:
        tc = ctx.enter_context(tile.TileContext(nc))
        big = ctx.enter_context(tc.tile_pool(name="big", bufs=1))
        sing = ctx.enter_context(tc.tile_pool(name="sing", bufs=1))
        wap = ctx.enter_context(tc.tile_pool(name="wap", bufs=2))
        abp = ctx.enter_context(tc.tile_pool(name="abp", bufs=2))
        p2p = ctx.enter_context(tc.tile_pool(name="p2p", bufs=2))
        trp = ctx.enter_context(tc.tile_pool(name="trp", bufs=1))
        t1p = ctx.enter_context(tc.tile_pool(name="t1p", bufs=2))
        sfp = ctx.enter_context(tc.tile_pool(name="sfp", bufs=2))
        csp = ctx.enter_context(tc.tile_pool(name="csp", bufs=2))
        sml = ctx.enter_context(tc.tile_pool(name="sml", bufs=1))
        psH = ctx.enter_context(tc.tile_pool(name="psH", bufs=4, space="PSUM"))
        ps0p = ctx.enter_context(tc.tile_pool(name="ps0p", bufs=1, space="PSUM"))
        psS = ctx.enter_context(tc.tile_pool(name="psS", bufs=1, space="PSUM"))
        psV = ctx.enter_context(tc.tile_pool(name="psV", bufs=1, space="PSUM"))

        hat = big.tile([128, FREE], f16)
        logits = sing.tile([128, CGJ], f16)
        inpT = sing.tile([128, C * BL], f16)
        biasl = sing.tile([128, C * J], f16)
        m8x = sing.tile([128, 8 * SN], f16)
        s8 = sing.tile([8, 128], f16)
        s32 = sing.tile([32, 512], f16)
        vrep0 = sing.tile([128, GJD], f16)
        vrep1 = sing.tile([128, GJD], f16)
        nc.sync.dma_start(out=inpT, in_=t_inpT[:])
        nc.sync.dma_start(out=biasl, in_=t_biasl[:])
        nc.sync.dma_start(out=m8x, in_=t_m8x[:])
        nc.sync.dma_start(out=s8, in_=t_s8[:])
        nc.sync.dma_start(out=s32, in_=t_s32[:])

        # ---------------- loop 1: s0 = (1/J) sum_i hat ----------------
        ps0 = ps0p.tile([BL, JD], fp32)
        for s in range(NSLAB):
            wa_s = wap.tile([128, SLAB * JD], f16, tag="wa")
            nc.sync.dma_start(out=wa_s,
                              in_=t_wa[:, s * SLAB * JD:(s + 1) * SLAB * JD])
            for cc in range(SLAB):
                c = s * SLAB + cc
                nc.tensor.matmul(ps0, inpT[:, c * BL:(c + 1) * BL],
                                 wa_s[:, cc * JD:(cc + 1) * JD],
                                 start=(c == 0), stop=(c == C - 1))

        # squash helpers -------------------------------------------------
        def squash(s_f32, P, nj, vname, vdt, sview):
            """v = squash(s). sview: [P, nj, 16] view builder for s-like."""
            sq = sml.tile([P, nj * D], fp32, tag=vname + "sq")
            nc.vector.tensor_mul(sq, s_f32, s_f32)
            s2 = sml.tile([P, nj], fp32, tag=vname + "s2")
            nc.vector.tensor_reduce(s2, sview(sq), axis=mybir.AxisListType.X,
                                    op=ADD)
            rt = sml.tile([P, nj], fp32, tag=vname + "rt")
            nc.scalar.sqrt(rt, s2)
            den = sml.tile([P, nj], fp32, tag=vname + "den")
            nc.vector.scalar_tensor_tensor(out=den, in0=s2, scalar=1.0,
                                           in1=rt, op0=ADD, op1=MUL)
            rden = sml.tile([P, nj], fp32, tag=vname + "rd")
            nc.vector.reciprocal(rden, den)
            sc = sml.tile([P, nj], fp32, tag=vname + "sc")
            nc.vector.tensor_mul(sc, s2, rden)
            v = sml.tile([P, nj * D], vdt, tag=vname)
            nc.vector.tensor_tensor(out=sview(v), in0=sview(s_f32),
                                    in1=bcast(sc[:, :], 1, D), op=MUL)
            return v

        # s0 is in (d, j) free order (wa column order is (d, j))
        def s0view(t):
            lst = [list(t.ap[0]), [1, J], [J, D]]
            return bass.AP(tensor=t.tensor, offset=t.offset, ap=lst)

        s0 = sml.tile([BL, JD], fp32, tag="s0")
        nc.scalar.mul(s0, ps0, 1.0 / J)
        v0h = squash(s0, BL, J, "v0", f16, s0view)

        # vrep0 [128, (g, d, j)]: vrep0[p, g] = v0h[g*8 + p%8]
        for half in range(2):
            pv = psV.tile([128, GJD // 2], fp32, tag="pv")
            for gh in range(2):
                g = half * 2 + gh
                nc.tensor.matmul(pv[:, gh * JD:(gh + 1) * JD],
                                 s32[:, g * 128:(g + 1) * 128], v0h,
                                 start=True, stop=True)
            nc.scalar.copy(vrep0[:, half * 320:(half + 1) * 320], pv)

        # ---------------- fused pass over hat ----------------
        def pass_block(s, vrep, pa, pb, first):
            hs = hat[:, s * SF:(s + 1) * SF]
            p2 = p2p.tile([128, SF], f16, tag="p2")
            nc.vector.tensor_tensor(
                out=p2.rearrange("p (c f) -> p c f", c=SLAB),
                in0=hs.rearrange("p (c f) -> p c f", c=SLAB),
                in1=bcast(vrep[:, :], 0, SLAB), op=MUL)
            p2v = p2.rearrange("p (n d j) -> p n d j", d=D, j=J)
            t1 = t1p.tile([128, SN * 8], f16, tag="t1")
            t1v = t1.rearrange("p (n d j) -> p n d j", d=8, j=J)
            nc.gpsimd.tensor_tensor(out=t1v, in0=p2v[:, :, 0:8, :],
                                    in1=p2v[:, :, 8:16, :], op=ADD)
            t2 = trp.tile([128, SN * 4], f16, tag="t2")
            t2v = t2.rearrange("p (n d j) -> p n d j", d=4, j=J)
            nc.vector.tensor_tensor(out=t2v, in0=t1v[:, :, 0:4, :],
                                    in1=t1v[:, :, 4:8, :], op=ADD)
            t3 = trp.tile([128, SN * 2], f16, tag="t3")
            t3v = t3.rearrange("p (n d j) -> p n d j", d=2, j=J)
            nc.vector.tensor_tensor(out=t3v, in0=t2v[:, :, 0:2, :],
                                    in1=t2v[:, :, 2:4, :], op=ADD)
            lsl = logits[:, s * SN:(s + 1) * SN]
            t4 = trp.tile([128, SN], f16, tag="t4")
            nc.vector.tensor_tensor(out=t4, in0=t3v[:, :, 0, :],
                                    in1=t3v[:, :, 1, :], op=ADD)
            if first:
                bsl = biasl[:, s * SLAB * J:(s + 1) * SLAB * J]
                nc.vector.tensor_tensor(
                    out=lsl.rearrange("p (c g j) -> p c g j", c=SLAB, g=G),
                    in0=t4.rearrange("p (c g j) -> p c g j", c=SLAB, g=G),
                    in1=bcast(bsl.rearrange("p (c j) -> p c j", c=SLAB), 1, G),
                    op=ADD)
            else:
                nc.vector.tensor_tensor(out=lsl, in0=lsl, in1=t4, op=ADD)
            ex = sfp.tile([128, SN], f16, tag="ex")
            nc.scalar.activation(ex, lsl, mybir.ActivationFunctionType.Exp)
            se = sml.tile([128, SN // J], fp32, tag="se")
            nc.vector.tensor_reduce(
                se, ex.rearrange("p (n j) -> p n j", j=J),
                axis=mybir.AxisListType.X, op=ADD)
            rse = sml.tile([128, SN // J], f16, tag="rse")
            with nc.allow_low_precision(reason="softmax denom f16"):
                nc.vector.reciprocal(rse, se)
            rsex = sfp.tile([128, SN], f16, tag="rsex")
            nc.scalar.copy(rsex.rearrange("p (n j) -> p n j", j=J),
                           bcast(rse[:, :], 1, J))
            ct = sfp.tile([128, SN], f16, tag="ct")
            nc.vector.tensor_tensor(out=ct, in0=ex, in1=rsex, op=MUL)
            csel = csp.tile([128, 8 * SN], f16, tag="cs")
            nc.vector.tensor_tensor(
                out=csel.rearrange("p (col n) -> p col n", n=SN),
                in0=bcast(ct[:, :], 0, 8),
                in1=m8x.rearrange("p (col n) -> p col n", n=SN), op=MUL)
            cv = csel.rearrange("p (col n) -> p n col", col=8)
            for cc in range(SLAB):
                c = s * SLAB + cc
                for g in range(G):
                    hm = hat[:, (c * G + g) * JD:(c * G + g + 1) * JD]
                    hmv = hm.rearrange("p (d j) -> p j d", j=J)
                    dst_t = pa if g < 2 else pb
                    for j in range(J):
                        n = (cc * G + g) * J + j
                        nc.tensor.matmul(
                            dst_t[:, ((g % 2) * J + j) * D:
                                  ((g % 2) * J + j + 1) * D],
                            cv[:, n, :], hmv[:, j, :],
                            start=(c == 0), stop=(c == C - 1))

        # ---------------- loop 2: hat build + pass 0 ----------------
        pa = psS.tile([8, GJD // 2], fp32, tag="pa")
        pb = psS.tile([8, GJD // 2], fp32, tag="pb")
        ev = [0]
        for s in range(NSLAB):
            wa2 = wap.tile([128, SLAB * JD], f16, tag="wa")
            nc.sync.dma_start(out=wa2,
                              in_=t_wa[:, s * SLAB * JD:(s + 1) * SLAB * JD])
            ab = abp.tile([128, SLAB * G * 128], f16, tag="ab")
            nc.sync.dma_start(
                out=ab, in_=t_ablk[:, s * SLAB * G * 128:
                                   (s + 1) * SLAB * G * 128])
            ph = None
            for cc in range(SLAB):
                for g in range(G):
                    k = (s * SLAB + cc) * G + g
                    slot = k % 3
                    if slot == 0:
                        ph = psH.tile([128, 3 * JD], fp32, tag="ph")
                    nc.tensor.matmul(
                        ph[:, slot * JD:(slot + 1) * JD],
                        ab[:, (cc * G + g) * 128:(cc * G + g + 1) * 128],
                        wa2[:, cc * JD:(cc + 1) * JD], start=True, stop=True)
                    if slot == 2:
                        dst = hat[:, (k - 2) * JD:(k + 1) * JD]
                        if ev[0] < DVE_EVACS:
                            nc.vector.tensor_copy(dst, ph)
                        else:
                            nc.scalar.copy(dst, ph)
                        ev[0] += 1
            pass_block(s, vrep0, pa, pb, True)

        # ---------------- iter 1: v1, then pass 1 ----------------
        def sgview(t):
            return t.rearrange("p (n d) -> p n d", d=D)

        s1 = sml.tile([8, GJD], fp32, tag="s1")
        nc.scalar.copy(s1[:, 0:320], pa)
        nc.scalar.copy(s1[:, 320:640], pb)
        v1h = squash(s1, 8, G * J, "vv", f16, sgview)
        # vrep1 [128, (g, d, j)] from v1h [8, (g, j, d)]
        v1v = v1h.rearrange("p (g j d) -> p g d j", g=G, j=J)
        for half in range(2):
            pv = psV.tile([128, GJD // 2], fp32, tag="pv")
            nc.tensor.matmul(pv, s8, v1v[:, half * 2:(half + 1) * 2],
                             start=True, stop=True)
            nc.scalar.copy(vrep1[:, half * 320:(half + 1) * 320], pv)

        pa2 = psS.tile([8, GJD // 2], fp32, tag="pa")
        pb2 = psS.tile([8, GJD // 2], fp32, tag="pb")
        for s in range(NSLAB):
            pass_block(s, vrep1, pa2, pb2, False)

        # ---------------- iter 2: v2 -> out ----------------
        s2 = sml.tile([8, GJD], fp32, tag="s2")
        nc.scalar.copy(s2[:, 0:320], pa2)
        nc.scalar.copy(s2[:, 320:640], pb2)
        v2 = squash(s2, 8, G * J, "vv", fp32, sgview)
        nc.sync.dma_start(out=t_out[:], in_=v2)

    nc.finalize()
    return nc


def _host_prep(x_full, W, bias):
    W = np.asarray(W, np.float32)
    wa = W.reshape(C, 16, J, E, D).transpose(1, 3, 0, 4, 2)  # [i16,e,c,d,j]
    wa = np.ascontiguousarray(wa.reshape(128, C * JD)).astype(F16)
    b2 = np.asarray(bias, np.float32).reshape(I, J)
    br = b2.reshape(C, 16, J).transpose(1, 0, 2)             # [i16,c,j]
    biasl = np.ascontiguousarray(
        np.broadcast_to(br[:, None], (16, 8, C, J)).reshape(128, C * J)
    ).astype(F16)
    m8x = np.zeros((128, 8, SN), F16)
    m8x[np.arange(128), np.arange(128) % 8, :] = 1
    m8x = m8x.reshape(128, 8 * SN)
    s8 = np.zeros((8, 128), F16)
    s8[np.arange(128) % 8, np.arange(128)] = 1
    s32 = np.zeros((32, 512), F16)
    for g in range(G):
        s32[g * 8 + np.arange(128) % 8, g * 128 + np.arange(128)] = 1
    maps = []
    for cl in range(NCORES):
        xl = np.asarray(x_full[cl * BL:(cl + 1) * BL], np.float32)
        inpT = xl.reshape(BL, C, 16, E).transpose(2, 3, 1, 0)  # [i16,e,c,b]
        inpT = np.ascontiguousarray(inpT.reshape(128, C * BL)).astype(F16)
        xr = xl.reshape(G, 8, C, 16, E)                        # [g,b8,c,i,e]
        z = np.zeros((16, 8, C, G, 16, 8), F16)
        for i in range(16):
            z[i, :, :, :, i, :] = xr[:, :, :, i, :].transpose(3, 2, 0, 1)
        ablk = z.reshape(128, C * G * 128)
        maps.append({"wa": wa, "inpT": inpT, "ablk": ablk, "biasl": biasl,
                     "m8x": m8x, "s8": s8, "s32": s32})
    return maps


_NC_CACHE = {}


def kernel(inputs, W, bias):
    from concourse import bass_utils

    if "nc" not in _NC_CACHE:
        _NC_CACHE["nc"] = _build_kernel()
    nc = _NC_CACHE["nc"]
    in_maps = _host_prep(inputs, W, bias)
    res = bass_utils.run_bass_kernel_spmd(nc, in_maps,
                                          core_ids=list(range(NCORES)))
    outs = []
    for r in res.results:
        v = r["out"].reshape(8, G, J, D).transpose(1, 0, 2, 3)  # [g,b8,j,d]
        outs.append(v.reshape(BL, J, D))
    return np.concatenate(outs, axis=0).astype(np.float32)


if __name__ == "__main__":
    import reference
    ins = reference.setup_inputs()
    ins = {k: np.asarray(v) for k, v in ins.items()}
    exp = np.asarray(reference.reference(**ins))
    got = kernel(**ins)
    err = np.abs(got - exp).max() / (np.abs(exp).max() + 1e-9)
    print("Relative error:", err)


# revision 8
# speedup vs baseline: 1.0480x; 1.0480x over previous
"""CapsuleLayer dynamic-routing kernel for 8 Trainium2 NeuronCores.

Data-parallel over batch (32 per core), W replicated. Per core:
  hat = einsum('bie,ijed->bijd') kept in SBUF f16, layout
  [p=(i%16)*8+(b%8), free=(c=i//16, g=b//8, d, j)].
  hat built by PE: stationary = host-built block-diag x matrices
  (ablk), moving = W chunks; s0 for routing iter 0 comes directly from
  inpT x W matmuls (uniform coupling).
Routing (3 iters, 2 fused passes):
  agreement  a=<hat,v>: DVE f16 mult + d-halving tree (2x mode).
  softmax    ACT exp + DVE reduce/recip.
  s = sum_i c*hat: per-(c,g,j) PE matmuls with c-selector stationaries
  (Csel[k=(i,b8), m=b8'] = c*delta), accumulated in PSUM -> no DVE mult.
"""

import sys
from contextlib import ExitStack

import numpy as np

sys.path.insert(0, "/opt/trn_rl_repo")

import ml_dtypes  # noqa: E402

F16 = ml_dtypes.float16 if hasattr(ml_dtypes, "float16") else np.float16

B, I, E = 256, 1152, 8
J, D = 10, 16
NCORES = 8
BL = B // NCORES          # 32 batches per core
C = I // 16               # 72 i-chunks of 16
G = BL // 8               # 4 b-groups of 8
JD = J * D                # 160
GJD = G * JD              # 640
CGJ = C * G * J           # 2880
FREE = C * G * JD         # 46080 free elems of hat per partition
SLAB = 9                  # c-chunks per slab
NSLAB = C // SLAB         # 8
SF = SLAB * GJD           # 5760 hat elems per slab per partition
SN = SLAB * G * J         # 360 (c,g,j) nodes per slab
NR = 3

# evac tiles handled by DVE (fills pre-v0 idle window); rest go to ACT
DVE_EVACS = 24


def _build_kernel():
    import concourse.bass as bass
    import concourse.bacc as bacc
    import concourse.tile as tile
    from concourse import mybir

    fp32 = mybir.dt.float32
    f16 = mybir.dt.float16
    ADD = mybir.AluOpType.add
    MUL = mybir.AluOpType.mult

    nc = bacc.Bacc("TRN2")
    t_wa = nc.dram_tensor("wa", [128, C * JD], f16, kind="ExternalInput")
    t_inpT = nc.dram_tensor("inpT", [128, C * BL], f16, kind="ExternalInput")
    t_ablk = nc.dram_tensor("ablk", [128, C * G * 128], f16,
                            kind="ExternalInput")
    t_biasl = nc.dram_tensor("biasl", [128, C * J], f16, kind="ExternalInput")
    t_m8x = nc.dram_tensor("m8x", [128, 8 * SN], f16, kind="ExternalInput")
    t_s8 = nc.dram_tensor("s8", [8, 128], f16, kind="ExternalInput")
    t_s32 = nc.dram_tensor("s32", [32, 512], f16, kind="ExternalInput")
    t_out = nc.dram_tensor("out", [8, GJD], fp32, kind="ExternalOutput")

    def bcast(ap, pos, n):
        """Insert a broadcast (step 0, count n) free dim at free-pos pos."""
        lst = [list(x) for x in ap.ap]
        lst.insert(1 + pos, [0, n])
        return bass.AP(tensor=ap.tensor, offset=ap.offset, ap=lst)

    with ExitStack() as ctx:
        tc = ctx.enter_context(tile.TileContext(nc))
        big = ctx.enter_context(tc.tile_pool(name="big", bufs=1))
        sing = ctx.enter_context(tc.tile_pool(name="sing", bufs=1))
        wap = ctx.enter_context(tc.tile_pool(name="wap", bufs=2))
        abp = ctx.enter_context(tc.tile_pool(name="abp", bufs=2))
        p2p = ctx.enter_context(tc.tile_pool(name="p2p", bufs=2))
        trp = ctx.enter_context(tc.tile_pool(name="trp", bufs=1))
        t1p = ctx.enter_context(tc.tile_pool(name="t1p", bufs=2))
        sfp = ctx.enter_context(tc.tile_pool(name="sfp", bufs=2))
        csp = ctx.enter_context(tc.tile_pool(name="csp", bufs=2))
        sml = ctx.enter_context(tc.tile_pool(name="sml", bufs=1))
        psH = ctx.enter_context(tc.tile_pool(name="psH", bufs=4, space="PSUM"))
        ps0p = ctx.enter_context(tc.tile_pool(name="ps0p", bufs=1, space="PSUM"))
        psS = ctx.enter_context(tc.tile_pool(name="psS", bufs=1, space="PSUM"))
        psV = ctx.enter_context(tc.tile_pool(name="psV", bufs=1, space="PSUM"))

        hat = big.tile([128, FREE], f16)
        logits = sing.tile([128, CGJ], f16)
        inpT = sing.tile([128, C * BL], f16)
        biasl = sing.tile([128, C * J], f16)
        m8x = sing.tile([128, 8 * SN], f16)
        s8 = sing.tile([8, 128], f16)
        s32 = sing.tile([32, 512], f16)
        vrep0 = sing.tile([128, GJD], f16)
        vrep1 = sing.tile([128, GJD], f16)
        nc.sync.dma_start(out=inpT, in_=t_inpT[:])
        nc.sync.dma_start(out=biasl, in_=t_biasl[:])
        nc.sync.dma_start(out=m8x, in_=t_m8x[:])
        nc.sync.dma_start(out=s8, in_=t_s8[:])
        nc.sync.dma_start(out=s32, in_=t_s32[:])

        # ---------------- loop 1: s0 = (1/J) sum_i hat ----------------
        ps0 = ps0p.tile([BL, JD], fp32)
        for s in range(NSLAB):
            wa_s = wap.tile([128, SLAB * JD], f16, tag="wa")
            nc.sync.dma_start(out=wa_s,
                              in_=t_wa[:, s * SLAB * JD:(s + 1) * SLAB * JD])
            for cc in range(SLAB):
                c = s * SLAB + cc
                nc.tensor.matmul(ps0, inpT[:, c * BL:(c + 1) * BL],
                                 wa_s[:, cc * JD:(cc + 1) * JD],
                                 start=(c == 0), stop=(c == C - 1))

        # squash helpers -------------------------------------------------
        def squash(s_f32, P, nj, vname, vdt, sview):
            """v = squash(s). sview: [P, nj, 16] view builder for s-like."""
            sq = sml.tile([P, nj * D], fp32, tag=vname + "sq")
            nc.vector.tensor_mul(sq, s_f32, s_f32)
            s2 = sml.tile([P, nj], fp32, tag=vname + "s2")
            nc.vector.tensor_reduce(s2, sview(sq), axis=mybir.AxisListType.X,
                                    op=ADD)
            rt = sml.tile([P, nj], fp32, tag=vname + "rt")
            nc.scalar.sqrt(rt, s2)
            den = sml.tile([P, nj], fp32, tag=vname + "den")
            nc.vector.scalar_tensor_tensor(out=den, in0=s2, scalar=1.0,
                                           in1=rt, op0=ADD, op1=MUL)
            rden = sml.tile([P, nj], fp32, tag=vname + "rd")
            nc.vector.reciprocal(rden, den)
            sc = sml.tile([P, nj], fp32, tag=vname + "sc")
            nc.vector.tensor_mul(sc, s2, rden)
            v = sml.tile([P, nj * D], vdt, tag=vname)
            nc.vector.tensor_tensor(out=sview(v), in0=sview(s_f32),
                                    in1=bcast(sc[:, :], 1, D), op=MUL)
            return v

        # s0 is in (d, j) free order (wa column order is (d, j))
        def s0view(t):
            lst = [list(t.ap[0]), [1, J], [J, D]]
            return bass.AP(tensor=t.tensor, offset=t.offset, ap=lst)

        s0 = sml.tile([BL, JD], fp32, tag="s0")
        nc.scalar.mul(s0, ps0, 1.0 / J)
        v0h = squash(s0, BL, J, "v0", f16, s0view)

        # vrep0 [128, (g, d, j)]: vrep0[p, g] = v0h[g*8 + p%8]
        for half in range(2):
            pv = psV.tile([128, GJD // 2], fp32, tag="pv")
            for gh in range(2):
                g = half * 2 + gh
                nc.tensor.matmul(pv[:, gh * JD:(gh + 1) * JD],
                                 s32[:, g * 128:(g + 1) * 128], v0h,
                                 start=True, stop=True)
            nc.scalar.copy(vrep0[:, half * 320:(half + 1) * 320], pv)

        # ---------------- fused pass over hat ----------------
        def stage1(s, vrep):
            hs = hat[:, s * SF:(s + 1) * SF]
            p2 = p2p.tile([128, SF], f16, tag="p2")
            nc.vector.tensor_tensor(
                out=p2.rearrange("p (c f) -> p c f", c=SLAB),
                in0=hs.rearrange("p (c f) -> p c f", c=SLAB),
                in1=bcast(vrep[:, :], 0, SLAB), op=MUL)
            p2v = p2.rearrange("p (n d j) -> p n d j", d=D, j=J)
            t1 = t1p.tile([128, SN * 8], f16, tag="t1")
            t1v = t1.rearrange("p (n d j) -> p n d j", d=8, j=J)
            nc.gpsimd.tensor_tensor(out=t1v, in0=p2v[:, :, 0:8, :],
                                    in1=p2v[:, :, 8:16, :], op=ADD)
            return t1v

        def stage2(s, t1v, pa, pb, first):
            t2 = trp.tile([128, SN * 4], f16, tag="t2")
            t2v = t2.rearrange("p (n d j) -> p n d j", d=4, j=J)
            nc.vector.tensor_tensor(out=t2v, in0=t1v[:, :, 0:4, :],
                                    in1=t1v[:, :, 4:8, :], op=ADD)
            t3 = trp.tile([128, SN * 2], f16, tag="t3")
            t3v = t3.rearrange("p (n d j) -> p n d j", d=2, j=J)
            nc.vector.tensor_tensor(out=t3v, in0=t2v[:, :, 0:2, :],
                                    in1=t2v[:, :, 2:4, :], op=ADD)
            lsl = logits[:, s * SN:(s + 1) * SN]
            t4 = trp.tile([128, SN], f16, tag="t4")
            nc.vector.tensor_tensor(out=t4, in0=t3v[:, :, 0, :],
                                    in1=t3v[:, :, 1, :], op=ADD)
            if first:
                bsl = biasl[:, s * SLAB * J:(s + 1) * SLAB * J]
                nc.vector.tensor_tensor(
                    out=lsl.rearrange("p (c g j) -> p c g j", c=SLAB, g=G),
                    in0=t4.rearrange("p (c g j) -> p c g j", c=SLAB, g=G),
                    in1=bcast(bsl.rearrange("p (c j) -> p c j", c=SLAB), 1, G),
                    op=ADD)
            else:
                nc.vector.tensor_tensor(out=lsl, in0=lsl, in1=t4, op=ADD)
            ex = sfp.tile([128, SN], f16, tag="ex")
            nc.scalar.activation(ex, lsl, mybir.ActivationFunctionType.Exp)
            se = sml.tile([128, SN // J], fp32, tag="se")
            nc.vector.tensor_reduce(
                se, ex.rearrange("p (n j) -> p n j", j=J),
                axis=mybir.AxisListType.X, op=ADD)
            rse = sml.tile([128, SN // J], f16, tag="rse")
            with nc.allow_low_precision(reason="softmax denom f16"):
                nc.vector.reciprocal(rse, se)
            rsex = sfp.tile([128, SN], f16, tag="rsex")
            nc.scalar.copy(rsex.rearrange("p (n j) -> p n j", j=J),
                           bcast(rse[:, :], 1, J))
            ct = sfp.tile([128, SN], f16, tag="ct")
            nc.vector.tensor_tensor(out=ct, in0=ex, in1=rsex, op=MUL)
            csel = csp.tile([128, 8 * SN], f16, tag="cs")
            nc.vector.tensor_tensor(
                out=csel.rearrange("p (col n) -> p col n", n=SN),
                in0=bcast(ct[:, :], 0, 8),
                in1=m8x.rearrange("p (col n) -> p col n", n=SN), op=MUL)
            cv = csel.rearrange("p (col n) -> p n col", col=8)
            for cc in range(SLAB):
                c = s * SLAB + cc
                for g in range(G):
                    hm = hat[:, (c * G + g) * JD:(c * G + g + 1) * JD]
                    hmv = hm.rearrange("p (d j) -> p j d", j=J)
                    dst_t = pa if g < 2 else pb
                    for j in range(J):
                        n = (cc * G + g) * J + j
                        nc.tensor.matmul(
                            dst_t[:, ((g % 2) * J + j) * D:
                                  ((g % 2) * J + j + 1) * D],
                            cv[:, n, :], hmv[:, j, :],
                            start=(c == 0), stop=(c == C - 1))

        # ---------------- loop 2: hat build + pass 0 ----------------
        pa = psS.tile([8, GJD // 2], fp32, tag="pa")
        pb = psS.tile([8, GJD // 2], fp32, tag="pb")
        ev = [0]
        pend = []
        for s in range(NSLAB):
            wa2 = wap.tile([128, SLAB * JD], f16, tag="wa")
            nc.sync.dma_start(out=wa2,
                              in_=t_wa[:, s * SLAB * JD:(s + 1) * SLAB * JD])
            ab = abp.tile([128, SLAB * G * 128], f16, tag="ab")
            nc.sync.dma_start(
                out=ab, in_=t_ablk[:, s * SLAB * G * 128:
                                   (s + 1) * SLAB * G * 128])
            ph = None
            for cc in range(SLAB):
                for g in range(G):
                    k = (s * SLAB + cc) * G + g
                    slot = k % 3
                    if slot == 0:
                        ph = psH.tile([128, 3 * JD], fp32, tag="ph")
                    nc.tensor.matmul(
                        ph[:, slot * JD:(slot + 1) * JD],
                        ab[:, (cc * G + g) * 128:(cc * G + g + 1) * 128],
                        wa2[:, cc * JD:(cc + 1) * JD], start=True, stop=True)
                    if slot == 2:
                        dst = hat[:, (k - 2) * JD:(k + 1) * JD]
                        if ev[0] < DVE_EVACS:
                            nc.vector.tensor_copy(dst, ph)
                        else:
                            nc.scalar.copy(dst, ph)
                        ev[0] += 1
            pend.append((s, stage1(s, vrep0)))
            if len(pend) == 2:
                ps_, t1v_ = pend.pop(0)
                stage2(ps_, t1v_, pa, pb, True)
        ps_, t1v_ = pend.pop(0)
        stage2(ps_, t1v_, pa, pb, True)

        # ---------------- iter 1: v1, then pass 1 ----------------
        def sgview(t):
            return t.rearrange("p (n d) -> p n d", d=D)

        s1 = sml.tile([8, GJD], fp32, tag="s1")
        nc.scalar.copy(s1[:, 0:320], pa)
        nc.scalar.copy(s1[:, 320:640], pb)
        v1h = squash(s1, 8, G * J, "vv", f16, sgview)
        # vrep1 [128, (g, d, j)] from v1h [8, (g, j, d)]
        v1v = v1h.rearrange("p (g j d) -> p g d j", g=G, j=J)
        for half in range(2):
            pv = psV.tile([128, GJD // 2], fp32, tag="pv")
            nc.tensor.matmul(pv, s8, v1v[:, half * 2:(half + 1) * 2],
                             start=True, stop=True)
            nc.scalar.copy(vrep1[:, half * 320:(half + 1) * 320], pv)

        pa2 = psS.tile([8, GJD // 2], fp32, tag="pa")
        pb2 = psS.tile([8, GJD // 2], fp32, tag="pb")
        pend = []
        for s in range(NSLAB):
            pend.append((s, stage1(s, vrep1)))
            if len(pend) == 2:
                ps_, t1v_ = pend.pop(0)
                stage2(ps_, t1v_, pa2, pb2, False)
        ps_, t1v_ = pend.pop(0)
        stage2(ps_, t1v_, pa2, pb2, False)

        # ---------------- iter 2: v2 -> out ----------------
        s2 = sml.tile([8, GJD], fp32, tag="s2")
        nc.scalar.copy(s2[:, 0:320], pa2)
        nc.scalar.copy(s2[:, 320:640], pb2)
        v2 = squash(s2, 8, G * J, "vv", fp32, sgview)
        nc.sync.dma_start(out=t_out[:], in_=v2)

    nc.finalize()
    return nc


def _host_prep(x_full, W, bias):
    W = np.asarray(W, np.float32)
    wa = W.reshape(C, 16, J, E, D).transpose(1, 3, 0, 4, 2)  # [i16,e,c,d,j]
    wa = np.ascontiguousarray(wa.reshape(128, C * JD)).astype(F16)
    b2 = np.asarray(bias, np.float32).reshape(I, J)
    br = b2.reshape(C, 16, J).transpose(1, 0, 2)             # [i16,c,j]
    biasl = np.ascontiguousarray(
        np.broadcast_to(br[:, None], (16, 8, C, J)).reshape(128, C * J)
    ).astype(F16)
    m8x = np.zeros((128, 8, SN), F16)
    m8x[np.arange(128), np.arange(128) % 8, :] = 1
    m8x = m8x.reshape(128, 8 * SN)
    s8 = np.zeros((8, 128), F16)
    s8[np.arange(128) % 8, np.arange(128)] = 1
    s32 = np.zeros((32, 512), F16)
    for g in range(G):
        s32[g * 8 + np.arange(128) % 8, g * 128 + np.arange(128)] = 1
    maps = []
    for cl in range(NCORES):
        xl = np.asarray(x_full[cl * BL:(cl + 1) * BL], np.float32)
        inpT = xl.reshape(BL, C, 16, E).transpose(2, 3, 1, 0)  # [i16,e,c,b]
        inpT = np.ascontiguousarray(inpT.reshape(128, C * BL)).astype(F16)
        xr = xl.reshape(G, 8, C, 16, E)                        # [g,b8,c,i,e]
        z = np.zeros((16, 8, C, G, 16, 8), F16)
        for i in range(16):
            z[i, :, :, :, i, :] = xr[:, :, :, i, :].transpose(3, 2, 0, 1)
        ablk = z.reshape(128, C * G * 128)
        maps.append({"wa": wa, "inpT": inpT, "ablk": ablk, "biasl": biasl,
                     "m8x": m8x, "s8": s8, "s32": s32})
    return maps


_NC_CACHE = {}


def kernel(inputs, W, bias):
    from concourse import bass_utils

    if "nc" not in _NC_CACHE:
        _NC_CACHE["nc"] = _build_kernel()
    nc = _NC_CACHE["nc"]
    in_maps = _host_prep(inputs, W, bias)
    res = bass_utils.run_bass_kernel_spmd(nc, in_maps,
                                          core_ids=list(range(NCORES)))
    outs = []
    for r in res.results:
        v = r["out"].reshape(8, G, J, D).transpose(1, 0, 2, 3)  # [g,b8,j,d]
        outs.append(v.reshape(BL, J, D))
    return np.concatenate(outs, axis=0).astype(np.float32)


if __name__ == "__main__":
    import reference
    ins = reference.setup_inputs()
    ins = {k: np.asarray(v) for k, v in ins.items()}
    exp = np.asarray(reference.reference(**ins))
    got = kernel(**ins)
    err = np.abs(got - exp).max() / (np.abs(exp).max() + 1e-9)
    print("Relative error:", err)


# revision 10
# speedup vs baseline: 1.0548x; 1.0064x over previous
"""CapsuleLayer dynamic-routing kernel for 8 Trainium2 NeuronCores.

Data-parallel over batch (32 per core), W replicated. Per core:
  hat = einsum('bie,ijed->bijd') kept in SBUF f16, layout
  [p=(i%16)*8+(b%8), free=(c=i//16, g=b//8, d, j)].
  hat built by PE: stationary = host-built block-diag x matrices
  (ablk), moving = W chunks; s0 for routing iter 0 comes directly from
  inpT x W matmuls (uniform coupling).
Routing (3 iters, 2 fused passes):
  agreement  a=<hat,v>: DVE f16 mult + d-halving tree (2x mode).
  softmax    ACT exp + DVE reduce/recip.
  s = sum_i c*hat: per-(c,g,j) PE matmuls with c-selector stationaries
  (Csel[k=(i,b8), m=b8'] = c*delta), accumulated in PSUM -> no DVE mult.
"""

import sys
from contextlib import ExitStack

import numpy as np

sys.path.insert(0, "/opt/trn_rl_repo")

import ml_dtypes  # noqa: E402

F16 = ml_dtypes.float16 if hasattr(ml_dtypes, "float16") else np.float16

B, I, E = 256, 1152, 8
J, D = 10, 16
NCORES = 8
BL = B // NCORES          # 32 batches per core
C = I // 16               # 72 i-chunks of 16
G = BL // 8               # 4 b-groups of 8
JD = J * D                # 160
GJD = G * JD              # 640
CGJ = C * G * J           # 2880
FREE = C * G * JD         # 46080 free elems of hat per partition
SLAB = 9                  # c-chunks per slab
NSLAB = C // SLAB         # 8
SF = SLAB * GJD           # 5760 hat elems per slab per partition
SN = SLAB * G * J         # 360 (c,g,j) nodes per slab
NR = 3

# evac tiles handled by DVE (fills pre-v0 idle window); rest go to ACT
DVE_EVACS = 24


def _build_kernel():
    import concourse.bass as bass
    import concourse.bacc as bacc
    import concourse.tile as tile
    from concourse import mybir

    fp32 = mybir.dt.float32
    f16 = mybir.dt.float16
    ADD = mybir.AluOpType.add
    MUL = mybir.AluOpType.mult

    nc = bacc.Bacc("TRN2")
    t_wa = nc.dram_tensor("wa", [128, C * JD], f16, kind="ExternalInput")
    t_inpT = nc.dram_tensor("inpT", [128, C * BL], f16, kind="ExternalInput")
    t_ablk = nc.dram_tensor("ablk", [128, C * G * 128], f16,
                            kind="ExternalInput")
    t_biasl = nc.dram_tensor("biasl", [128, C * J], f16, kind="ExternalInput")
    t_m8x = nc.dram_tensor("m8x", [128, 8 * SN], f16, kind="ExternalInput")
    t_s8 = nc.dram_tensor("s8", [8, 128], f16, kind="ExternalInput")
    t_s32 = nc.dram_tensor("s32", [32, 512], f16, kind="ExternalInput")
    t_out = nc.dram_tensor("out", [8, GJD], fp32, kind="ExternalOutput")

    def bcast(ap, pos, n):
        """Insert a broadcast (step 0, count n) free dim at free-pos pos."""
        lst = [list(x) for x in ap.ap]
        lst.insert(1 + pos, [0, n])
        return bass.AP(tensor=ap.tensor, offset=ap.offset, ap=lst)

    with ExitStack() as ctx:
        tc = ctx.enter_context(tile.TileContext(nc))
        big = ctx.enter_context(tc.tile_pool(name="big", bufs=1))
        sing = ctx.enter_context(tc.tile_pool(name="sing", bufs=1))
        wap = ctx.enter_context(tc.tile_pool(name="wap", bufs=2))
        abp = ctx.enter_context(tc.tile_pool(name="abp", bufs=2))
        p2p = ctx.enter_context(tc.tile_pool(name="p2p", bufs=2))
        trp = ctx.enter_context(tc.tile_pool(name="trp", bufs=1))
        t1p = ctx.enter_context(tc.tile_pool(name="t1p", bufs=3))
        sfp = ctx.enter_context(tc.tile_pool(name="sfp", bufs=2))
        csp = ctx.enter_context(tc.tile_pool(name="csp", bufs=2))
        # deeper pipeline lag
        sml = ctx.enter_context(tc.tile_pool(name="sml", bufs=1))
        psH = ctx.enter_context(tc.tile_pool(name="psH", bufs=4, space="PSUM"))
        ps0p = ctx.enter_context(tc.tile_pool(name="ps0p", bufs=1, space="PSUM"))
        psS = ctx.enter_context(tc.tile_pool(name="psS", bufs=1, space="PSUM"))
        psV = ctx.enter_context(tc.tile_pool(name="psV", bufs=1, space="PSUM"))

        hat = big.tile([128, FREE], f16)
        logits = sing.tile([128, CGJ], f16)
        inpT = sing.tile([128, C * BL], f16)
        biasl = sing.tile([128, C * J], f16)
        m8x = sing.tile([128, 8 * SN], f16)
        s8 = sing.tile([8, 128], f16)
        s32 = sing.tile([32, 512], f16)
        vrep0 = sing.tile([128, GJD], f16)
        vrep1 = sing.tile([128, GJD], f16)
        nc.sync.dma_start(out=inpT, in_=t_inpT[:])
        nc.sync.dma_start(out=biasl, in_=t_biasl[:])
        nc.sync.dma_start(out=m8x, in_=t_m8x[:])
        nc.sync.dma_start(out=s8, in_=t_s8[:])
        nc.sync.dma_start(out=s32, in_=t_s32[:])

        # ---------------- loop 1: s0 = (1/J) sum_i hat ----------------
        ps0 = ps0p.tile([BL, JD], fp32)
        for s in range(NSLAB):
            wa_s = wap.tile([128, SLAB * JD], f16, tag="wa")
            nc.sync.dma_start(out=wa_s,
                              in_=t_wa[:, s * SLAB * JD:(s + 1) * SLAB * JD])
            for cc in range(SLAB):
                c = s * SLAB + cc
                nc.tensor.matmul(ps0, inpT[:, c * BL:(c + 1) * BL],
                                 wa_s[:, cc * JD:(cc + 1) * JD],
                                 start=(c == 0), stop=(c == C - 1))

        # squash helpers -------------------------------------------------
        def squash(s_f32, P, nj, vname, vdt, sview):
            """v = squash(s). sview: [P, nj, 16] view builder for s-like."""
            sq = sml.tile([P, nj * D], fp32, tag=vname + "sq")
            nc.vector.tensor_mul(sq, s_f32, s_f32)
            s2 = sml.tile([P, nj], fp32, tag=vname + "s2")
            nc.vector.tensor_reduce(s2, sview(sq), axis=mybir.AxisListType.X,
                                    op=ADD)
            rt = sml.tile([P, nj], fp32, tag=vname + "rt")
            nc.scalar.sqrt(rt, s2)
            den = sml.tile([P, nj], fp32, tag=vname + "den")
            nc.vector.scalar_tensor_tensor(out=den, in0=s2, scalar=1.0,
                                           in1=rt, op0=ADD, op1=MUL)
            rden = sml.tile([P, nj], fp32, tag=vname + "rd")
            nc.vector.reciprocal(rden, den)
            sc = sml.tile([P, nj], fp32, tag=vname + "sc")
            nc.vector.tensor_mul(sc, s2, rden)
            v = sml.tile([P, nj * D], vdt, tag=vname)
            nc.vector.tensor_tensor(out=sview(v), in0=sview(s_f32),
                                    in1=bcast(sc[:, :], 1, D), op=MUL)
            return v

        # s0 is in (d, j) free order (wa column order is (d, j))
        def s0view(t):
            lst = [list(t.ap[0]), [1, J], [J, D]]
            return bass.AP(tensor=t.tensor, offset=t.offset, ap=lst)

        s0 = sml.tile([BL, JD], fp32, tag="s0")
        nc.scalar.mul(s0, ps0, 1.0 / J)
        v0h = squash(s0, BL, J, "v0", f16, s0view)

        # vrep0 [128, (g, d, j)]: vrep0[p, g] = v0h[g*8 + p%8]
        for half in range(2):
            pv = psV.tile([128, GJD // 2], fp32, tag="pv")
            for gh in range(2):
                g = half * 2 + gh
                nc.tensor.matmul(pv[:, gh * JD:(gh + 1) * JD],
                                 s32[:, g * 128:(g + 1) * 128], v0h,
                                 start=True, stop=True)
            nc.scalar.copy(vrep0[:, half * 320:(half + 1) * 320], pv)

        # ---------------- fused pass over hat ----------------
        def stage1(s, vrep):
            hs = hat[:, s * SF:(s + 1) * SF]
            p2 = p2p.tile([128, SF], f16, tag="p2")
            nc.vector.tensor_tensor(
                out=p2.rearrange("p (c f) -> p c f", c=SLAB),
                in0=hs.rearrange("p (c f) -> p c f", c=SLAB),
                in1=bcast(vrep[:, :], 0, SLAB), op=MUL)
            p2v = p2.rearrange("p (n d j) -> p n d j", d=D, j=J)
            t1 = t1p.tile([128, SN * 8], f16, tag="t1")
            t1v = t1.rearrange("p (n d j) -> p n d j", d=8, j=J)
            nc.gpsimd.tensor_tensor(out=t1v, in0=p2v[:, :, 0:8, :],
                                    in1=p2v[:, :, 8:16, :], op=ADD)
            return t1v

        def stage2(s, t1v, pa, pb, first):
            t2 = trp.tile([128, SN * 4], f16, tag="t2")
            t2v = t2.rearrange("p (n d j) -> p n d j", d=4, j=J)
            nc.vector.tensor_tensor(out=t2v, in0=t1v[:, :, 0:4, :],
                                    in1=t1v[:, :, 4:8, :], op=ADD)
            t3 = trp.tile([128, SN * 2], f16, tag="t3")
            t3v = t3.rearrange("p (n d j) -> p n d j", d=2, j=J)
            nc.vector.tensor_tensor(out=t3v, in0=t2v[:, :, 0:2, :],
                                    in1=t2v[:, :, 2:4, :], op=ADD)
            lsl = logits[:, s * SN:(s + 1) * SN]
            t4 = trp.tile([128, SN], f16, tag="t4")
            nc.vector.tensor_tensor(out=t4, in0=t3v[:, :, 0, :],
                                    in1=t3v[:, :, 1, :], op=ADD)
            if first:
                bsl = biasl[:, s * SLAB * J:(s + 1) * SLAB * J]
                nc.vector.tensor_tensor(
                    out=lsl.rearrange("p (c g j) -> p c g j", c=SLAB, g=G),
                    in0=t4.rearrange("p (c g j) -> p c g j", c=SLAB, g=G),
                    in1=bcast(bsl.rearrange("p (c j) -> p c j", c=SLAB), 1, G),
                    op=ADD)
            else:
                nc.vector.tensor_tensor(out=lsl, in0=lsl, in1=t4, op=ADD)
            ex = sfp.tile([128, SN], f16, tag="ex")
            nc.scalar.activation(ex, lsl, mybir.ActivationFunctionType.Exp)
            se = sml.tile([128, SN // J], fp32, tag="se")
            nc.vector.tensor_reduce(
                se, ex.rearrange("p (n j) -> p n j", j=J),
                axis=mybir.AxisListType.X, op=ADD)
            rse = sml.tile([128, SN // J], f16, tag="rse")
            with nc.allow_low_precision(reason="softmax denom f16"):
                nc.vector.reciprocal(rse, se)
            rsex = sfp.tile([128, SN], f16, tag="rsex")
            nc.scalar.copy(rsex.rearrange("p (n j) -> p n j", j=J),
                           bcast(rse[:, :], 1, J))
            ct = sfp.tile([128, SN], f16, tag="ct")
            nc.vector.tensor_tensor(out=ct, in0=ex, in1=rsex, op=MUL)
            csel = csp.tile([128, 8 * SN], f16, tag="cs")
            nc.vector.tensor_tensor(
                out=csel.rearrange("p (col n) -> p col n", n=SN),
                in0=bcast(ct[:, :], 0, 8),
                in1=m8x.rearrange("p (col n) -> p col n", n=SN), op=MUL)
            cv = csel.rearrange("p (col n) -> p n col", col=8)
            for cc in range(SLAB):
                c = s * SLAB + cc
                for g in range(G):
                    hm = hat[:, (c * G + g) * JD:(c * G + g + 1) * JD]
                    hmv = hm.rearrange("p (d j) -> p j d", j=J)
                    dst_t = pa if g < 2 else pb
                    for j in range(J):
                        n = (cc * G + g) * J + j
                        nc.tensor.matmul(
                            dst_t[:, ((g % 2) * J + j) * D:
                                  ((g % 2) * J + j + 1) * D],
                            cv[:, n, :], hmv[:, j, :],
                            start=(c == 0), stop=(c == C - 1))

        # ---------------- loop 2: hat build + pass 0 ----------------
        pa = psS.tile([8, GJD // 2], fp32, tag="pa")
        pb = psS.tile([8, GJD // 2], fp32, tag="pb")
        ev = [0]
        pend = []
        for s in range(NSLAB):
            wa2 = wap.tile([128, SLAB * JD], f16, tag="wa")
            nc.sync.dma_start(out=wa2,
                              in_=t_wa[:, s * SLAB * JD:(s + 1) * SLAB * JD])
            ab = abp.tile([128, SLAB * G * 128], f16, tag="ab")
            nc.sync.dma_start(
                out=ab, in_=t_ablk[:, s * SLAB * G * 128:
                                   (s + 1) * SLAB * G * 128])
            ph = None
            for cc in range(SLAB):
                for g in range(G):
                    k = (s * SLAB + cc) * G + g
                    slot = k % 3
                    if slot == 0:
                        ph = psH.tile([128, 3 * JD], fp32, tag="ph")
                    nc.tensor.matmul(
                        ph[:, slot * JD:(slot + 1) * JD],
                        ab[:, (cc * G + g) * 128:(cc * G + g + 1) * 128],
                        wa2[:, cc * JD:(cc + 1) * JD], start=True, stop=True)
                    if slot == 2:
                        dst = hat[:, (k - 2) * JD:(k + 1) * JD]
                        if ev[0] < DVE_EVACS:
                            nc.vector.tensor_copy(dst, ph)
                        else:
                            nc.scalar.copy(dst, ph)
                        ev[0] += 1
            pend.append((s, stage1(s, vrep0)))
            if len(pend) == 3:
                ps_, t1v_ = pend.pop(0)
                stage2(ps_, t1v_, pa, pb, True)
        while pend:
            ps_, t1v_ = pend.pop(0)
            stage2(ps_, t1v_, pa, pb, True)

        # ---------------- iter 1: v1, then pass 1 ----------------
        def sgview(t):
            return t.rearrange("p (n d) -> p n d", d=D)

        s1 = sml.tile([8, GJD], fp32, tag="s1")
        nc.scalar.copy(s1[:, 0:320], pa)
        nc.scalar.copy(s1[:, 320:640], pb)
        v1h = squash(s1, 8, G * J, "vv", f16, sgview)
        # vrep1 [128, (g, d, j)] from v1h [8, (g, j, d)]
        v1v = v1h.rearrange("p (g j d) -> p g d j", g=G, j=J)
        for half in range(2):
            pv = psV.tile([128, GJD // 2], fp32, tag="pv")
            nc.tensor.matmul(pv, s8, v1v[:, half * 2:(half + 1) * 2],
                             start=True, stop=True)
            nc.scalar.copy(vrep1[:, half * 320:(half + 1) * 320], pv)

        pa2 = psS.tile([8, GJD // 2], fp32, tag="pa")
        pb2 = psS.tile([8, GJD // 2], fp32, tag="pb")
        pend = []
        for s in range(NSLAB):
            pend.append((s, stage1(s, vrep1)))
            if len(pend) == 3:
                ps_, t1v_ = pend.pop(0)
                stage2(ps_, t1v_, pa2, pb2, False)
        while pend:
            ps_, t1v_ = pend.pop(0)
            stage2(ps_, t1v_, pa2, pb2, False)

        # ---------------- iter 2: v2 -> out ----------------
        s2 = sml.tile([8, GJD], fp32, tag="s2")
        nc.scalar.copy(s2[:, 0:320], pa2)
        nc.scalar.copy(s2[:, 320:640], pb2)
        v2 = squash(s2, 8, G * J, "vv", fp32, sgview)
        nc.sync.dma_start(out=t_out[:], in_=v2)

    nc.finalize()
    return nc


def _host_prep(x_full, W, bias):
    W = np.asarray(W, np.float32)
    wa = W.reshape(C, 16, J, E, D).transpose(1, 3, 0, 4, 2)  # [i16,e,c,d,j]
    wa = np.ascontiguousarray(wa.reshape(128, C * JD)).astype(F16)
    b2 = np.asarray(bias, np.float32).reshape(I, J)
    br = b2.reshape(C, 16, J).transpose(1, 0, 2)             # [i16,c,j]
    biasl = np.ascontiguousarray(
        np.broadcast_to(br[:, None], (16, 8, C, J)).reshape(128, C * J)
    ).astype(F16)
    m8x = np.zeros((128, 8, SN), F16)
    m8x[np.arange(128), np.arange(128) % 8, :] = 1
    m8x = m8x.reshape(128, 8 * SN)
    s8 = np.zeros((8, 128), F16)
    s8[np.arange(128) % 8, np.arange(128)] = 1
    s32 = np.zeros((32, 512), F16)
    for g in range(G):
        s32[g * 8 + np.arange(128) % 8, g * 128 + np.arange(128)] = 1
    maps = []
    for cl in range(NCORES):
        xl = np.asarray(x_full[cl * BL:(cl + 1) * BL], np.float32)
        inpT = xl.reshape(BL, C, 16, E).transpose(2, 3, 1, 0)  # [i16,e,c,b]
        inpT = np.ascontiguousarray(inpT.reshape(128, C * BL)).astype(F16)
        xr = xl.reshape(G, 8, C, 16, E)                        # [g,b8,c,i,e]
        z = np.zeros((16, 8, C, G, 16, 8), F16)
        for i in range(16):
            z[i, :, :, :, i, :] = xr[:, :, :, i, :].transpose(3, 2, 0, 1)
        ablk = z.reshape(128, C * G * 128)
        maps.append({"wa": wa, "inpT": inpT, "ablk": ablk, "biasl": biasl,
                     "m8x": m8x, "s8": s8, "s32": s32})
    return maps


_NC_CACHE = {}


def kernel(inputs, W, bias):
    from concourse import bass_utils

    if "nc" not in _NC_CACHE:
        _NC_CACHE["nc"] = _build_kernel()
    nc = _NC_CACHE["nc"]
    in_maps = _host_prep(inputs, W, bias)
    res = bass_utils.run_bass_kernel_spmd(nc, in_maps,
                                          core_ids=list(range(NCORES)))
    outs = []
    for r in res.results:
        v = r["out"].reshape(8, G, J, D).transpose(1, 0, 2, 3)  # [g,b8,j,d]
        outs.append(v.reshape(BL, J, D))
    return np.concatenate(outs, axis=0).astype(np.float32)


if __name__ == "__main__":
    import reference
    ins = reference.setup_inputs()
    ins = {k: np.asarray(v) for k, v in ins.items()}
    exp = np.asarray(reference.reference(**ins))
    got = kernel(**ins)
    err = np.abs(got - exp).max() / (np.abs(exp).max() + 1e-9)
    print("Relative error:", err)


# revision 12
# speedup vs baseline: 1.0551x; 1.0003x over previous
"""CapsuleLayer dynamic-routing kernel for 8 Trainium2 NeuronCores.

Data-parallel over batch (32 per core), W replicated. Per core:
  hat = einsum('bie,ijed->bijd') kept in SBUF f16, layout
  [p=(i%16)*8+(b%8), free=(c=i//16, g=b//8, d, j)].
  hat built by PE: stationary = host-built block-diag x matrices
  (ablk), moving = W chunks; s0 for routing iter 0 comes directly from
  inpT x W matmuls (uniform coupling).
Routing (3 iters, 2 fused passes):
  agreement  a=<hat,v>: DVE f16 mult + d-halving tree (2x mode).
  softmax    ACT exp + DVE reduce/recip.
  s = sum_i c*hat: per-(c,g,j) PE matmuls with c-selector stationaries
  (Csel[k=(i,b8), m=b8'] = c*delta), accumulated in PSUM -> no DVE mult.
"""

import sys
from contextlib import ExitStack

import numpy as np

sys.path.insert(0, "/opt/trn_rl_repo")

import ml_dtypes  # noqa: E402

F16 = ml_dtypes.float16 if hasattr(ml_dtypes, "float16") else np.float16

B, I, E = 256, 1152, 8
J, D = 10, 16
NCORES = 8
BL = B // NCORES          # 32 batches per core
C = I // 16               # 72 i-chunks of 16
G = BL // 8               # 4 b-groups of 8
JD = J * D                # 160
GJD = G * JD              # 640
CGJ = C * G * J           # 2880
FREE = C * G * JD         # 46080 free elems of hat per partition
SLAB = 9                  # c-chunks per slab
NSLAB = C // SLAB         # 8
SF = SLAB * GJD           # 5760 hat elems per slab per partition
SN = SLAB * G * J         # 360 (c,g,j) nodes per slab
NR = 3

# evac groups handled by DVE (fills pre-v0 idle window); rest go to ACT
DVE_EVACS = 2


def _build_kernel():
    import concourse.bass as bass
    import concourse.bacc as bacc
    import concourse.tile as tile
    from concourse import mybir

    fp32 = mybir.dt.float32
    f16 = mybir.dt.float16
    ADD = mybir.AluOpType.add
    MUL = mybir.AluOpType.mult

    nc = bacc.Bacc("TRN2")
    t_wa = nc.dram_tensor("wa", [128, C * JD], f16, kind="ExternalInput")
    t_inpT = nc.dram_tensor("inpT", [128, C * BL], f16, kind="ExternalInput")
    t_ablk = nc.dram_tensor("ablk", [128, C * G * 128], f16,
                            kind="ExternalInput")
    t_biasl = nc.dram_tensor("biasl", [128, C * J], f16, kind="ExternalInput")
    t_m8x = nc.dram_tensor("m8x", [128, 8 * SN], f16, kind="ExternalInput")
    t_s8 = nc.dram_tensor("s8", [8, 128], f16, kind="ExternalInput")
    t_s32 = nc.dram_tensor("s32", [32, 512], f16, kind="ExternalInput")
    t_out = nc.dram_tensor("out", [8, GJD], fp32, kind="ExternalOutput")

    def bcast(ap, pos, n):
        """Insert a broadcast (step 0, count n) free dim at free-pos pos."""
        lst = [list(x) for x in ap.ap]
        lst.insert(1 + pos, [0, n])
        return bass.AP(tensor=ap.tensor, offset=ap.offset, ap=lst)

    with ExitStack() as ctx:
        tc = ctx.enter_context(tile.TileContext(nc))
        big = ctx.enter_context(tc.tile_pool(name="big", bufs=1))
        sing = ctx.enter_context(tc.tile_pool(name="sing", bufs=1))
        wap = ctx.enter_context(tc.tile_pool(name="wap", bufs=2))
        abp = ctx.enter_context(tc.tile_pool(name="abp", bufs=2))
        p2p = ctx.enter_context(tc.tile_pool(name="p2p", bufs=2))
        trp = ctx.enter_context(tc.tile_pool(name="trp", bufs=1))
        t1p = ctx.enter_context(tc.tile_pool(name="t1p", bufs=3))
        sfp = ctx.enter_context(tc.tile_pool(name="sfp", bufs=2))
        csp = ctx.enter_context(tc.tile_pool(name="csp", bufs=2))
        # deeper pipeline lag
        sml = ctx.enter_context(tc.tile_pool(name="sml", bufs=1))
        psH = ctx.enter_context(tc.tile_pool(name="psH", bufs=2, space="PSUM"))
        ps0p = ctx.enter_context(tc.tile_pool(name="ps0p", bufs=1, space="PSUM"))
        psS = ctx.enter_context(tc.tile_pool(name="psS", bufs=1, space="PSUM"))
        psV = ctx.enter_context(tc.tile_pool(name="psV", bufs=1, space="PSUM"))

        hat = big.tile([128, FREE], f16)
        logits = sing.tile([128, CGJ], f16)
        inpT = sing.tile([128, C * BL], f16)
        biasl = sing.tile([128, C * J], f16)
        m8x = sing.tile([128, 8 * SN], f16)
        s8 = sing.tile([8, 128], f16)
        s32 = sing.tile([32, 512], f16)
        vrep0 = sing.tile([128, GJD], f16)
        vrep1 = sing.tile([128, GJD], f16)
        nc.sync.dma_start(out=inpT, in_=t_inpT[:])
        nc.sync.dma_start(out=biasl, in_=t_biasl[:])
        nc.sync.dma_start(out=m8x, in_=t_m8x[:])
        nc.sync.dma_start(out=s8, in_=t_s8[:])
        nc.sync.dma_start(out=s32, in_=t_s32[:])

        # ---------------- loop 1: s0 = (1/J) sum_i hat ----------------
        ps0 = ps0p.tile([BL, JD], fp32)
        for s in range(NSLAB):
            wa_s = wap.tile([128, SLAB * JD], f16, tag="wa")
            nc.sync.dma_start(out=wa_s,
                              in_=t_wa[:, s * SLAB * JD:(s + 1) * SLAB * JD])
            for cc in range(SLAB):
                c = s * SLAB + cc
                nc.tensor.matmul(ps0, inpT[:, c * BL:(c + 1) * BL],
                                 wa_s[:, cc * JD:(cc + 1) * JD],
                                 start=(c == 0), stop=(c == C - 1))

        # squash helpers -------------------------------------------------
        def squash(s_f32, P, nj, vname, vdt, sview):
            """v = squash(s). sview: [P, nj, 16] view builder for s-like."""
            sq = sml.tile([P, nj * D], fp32, tag=vname + "sq")
            nc.vector.tensor_mul(sq, s_f32, s_f32)
            s2 = sml.tile([P, nj], fp32, tag=vname + "s2")
            nc.vector.tensor_reduce(s2, sview(sq), axis=mybir.AxisListType.X,
                                    op=ADD)
            rt = sml.tile([P, nj], fp32, tag=vname + "rt")
            nc.scalar.sqrt(rt, s2)
            den = sml.tile([P, nj], fp32, tag=vname + "den")
            nc.vector.scalar_tensor_tensor(out=den, in0=s2, scalar=1.0,
                                           in1=rt, op0=ADD, op1=MUL)
            rden = sml.tile([P, nj], fp32, tag=vname + "rd")
            nc.vector.reciprocal(rden, den)
            sc = sml.tile([P, nj], fp32, tag=vname + "sc")
            nc.vector.tensor_mul(sc, s2, rden)
            v = sml.tile([P, nj * D], vdt, tag=vname)
            nc.vector.tensor_tensor(out=sview(v), in0=sview(s_f32),
                                    in1=bcast(sc[:, :], 1, D), op=MUL)
            return v

        # s0 is in (d, j) free order (wa column order is (d, j))
        def s0view(t):
            lst = [list(t.ap[0]), [1, J], [J, D]]
            return bass.AP(tensor=t.tensor, offset=t.offset, ap=lst)

        s0 = sml.tile([BL, JD], fp32, tag="s0")
        nc.scalar.mul(s0, ps0, 1.0 / J)
        v0h = squash(s0, BL, J, "v0", f16, s0view)

        # vrep0 [128, (g, d, j)]: vrep0[p, g] = v0h[g*8 + p%8]
        for half in range(2):
            pv = psV.tile([128, GJD // 2], fp32, tag="pv")
            for gh in range(2):
                g = half * 2 + gh
                nc.tensor.matmul(pv[:, gh * JD:(gh + 1) * JD],
                                 s32[:, g * 128:(g + 1) * 128], v0h,
                                 start=True, stop=True)
            nc.scalar.copy(vrep0[:, half * 320:(half + 1) * 320], pv)

        # ---------------- fused pass over hat ----------------
        def stage1(s, vrep):
            hs = hat[:, s * SF:(s + 1) * SF]
            p2 = p2p.tile([128, SF], f16, tag="p2")
            nc.vector.tensor_tensor(
                out=p2.rearrange("p (c f) -> p c f", c=SLAB),
                in0=hs.rearrange("p (c f) -> p c f", c=SLAB),
                in1=bcast(vrep[:, :], 0, SLAB), op=MUL)
            p2v = p2.rearrange("p (n d j) -> p n d j", d=D, j=J)
            t1 = t1p.tile([128, SN * 8], f16, tag="t1")
            t1v = t1.rearrange("p (n d j) -> p n d j", d=8, j=J)
            nc.gpsimd.tensor_tensor(out=t1v, in0=p2v[:, :, 0:8, :],
                                    in1=p2v[:, :, 8:16, :], op=ADD)
            return t1v

        def stage2(s, t1v, pa, pb, first):
            t2 = trp.tile([128, SN * 4], f16, tag="t2")
            t2v = t2.rearrange("p (n d j) -> p n d j", d=4, j=J)
            nc.vector.tensor_tensor(out=t2v, in0=t1v[:, :, 0:4, :],
                                    in1=t1v[:, :, 4:8, :], op=ADD)
            t3 = trp.tile([128, SN * 2], f16, tag="t3")
            t3v = t3.rearrange("p (n d j) -> p n d j", d=2, j=J)
            nc.vector.tensor_tensor(out=t3v, in0=t2v[:, :, 0:2, :],
                                    in1=t2v[:, :, 2:4, :], op=ADD)
            lsl = logits[:, s * SN:(s + 1) * SN]
            t4 = trp.tile([128, SN], f16, tag="t4")
            nc.vector.tensor_tensor(out=t4, in0=t3v[:, :, 0, :],
                                    in1=t3v[:, :, 1, :], op=ADD)
            if first:
                bsl = biasl[:, s * SLAB * J:(s + 1) * SLAB * J]
                nc.vector.tensor_tensor(
                    out=lsl.rearrange("p (c g j) -> p c g j", c=SLAB, g=G),
                    in0=t4.rearrange("p (c g j) -> p c g j", c=SLAB, g=G),
                    in1=bcast(bsl.rearrange("p (c j) -> p c j", c=SLAB), 1, G),
                    op=ADD)
            else:
                nc.vector.tensor_tensor(out=lsl, in0=lsl, in1=t4, op=ADD)
            ex = sfp.tile([128, SN], f16, tag="ex")
            nc.scalar.activation(ex, lsl, mybir.ActivationFunctionType.Exp)
            se = sml.tile([128, SN // J], fp32, tag="se")
            nc.vector.tensor_reduce(
                se, ex.rearrange("p (n j) -> p n j", j=J),
                axis=mybir.AxisListType.X, op=ADD)
            rse = sml.tile([128, SN // J], f16, tag="rse")
            with nc.allow_low_precision(reason="softmax denom f16"):
                nc.vector.reciprocal(rse, se)
            rsex = sfp.tile([128, SN], f16, tag="rsex")
            if first:
                nc.gpsimd.tensor_copy(rsex.rearrange("p (n j) -> p n j", j=J),
                                      bcast(rse[:, :], 1, J))
            else:
                nc.scalar.copy(rsex.rearrange("p (n j) -> p n j", j=J),
                               bcast(rse[:, :], 1, J))
            ct = sfp.tile([128, SN], f16, tag="ct")
            nc.vector.tensor_tensor(out=ct, in0=ex, in1=rsex, op=MUL)
            csel = csp.tile([128, 8 * SN], f16, tag="cs")
            nc.vector.tensor_tensor(
                out=csel.rearrange("p (col n) -> p col n", n=SN),
                in0=bcast(ct[:, :], 0, 8),
                in1=m8x.rearrange("p (col n) -> p col n", n=SN), op=MUL)
            cv = csel.rearrange("p (col n) -> p n col", col=8)
            for cc in range(SLAB):
                c = s * SLAB + cc
                for g in range(G):
                    hm = hat[:, (c * G + g) * JD:(c * G + g + 1) * JD]
                    hmv = hm.rearrange("p (d j) -> p j d", j=J)
                    dst_t = pa if g < 2 else pb
                    for j in range(J):
                        n = (cc * G + g) * J + j
                        nc.tensor.matmul(
                            dst_t[:, ((g % 2) * J + j) * D:
                                  ((g % 2) * J + j + 1) * D],
                            cv[:, n, :], hmv[:, j, :],
                            start=(c == 0), stop=(c == C - 1))

        # ---------------- loop 2: hat build + pass 0 ----------------
        pa = psS.tile([8, GJD // 2], fp32, tag="pa")
        pb = psS.tile([8, GJD // 2], fp32, tag="pb")
        ev = [0]
        pend = []
        for s in range(NSLAB):
            wa2 = wap.tile([128, SLAB * JD], f16, tag="wa")
            nc.sync.dma_start(out=wa2,
                              in_=t_wa[:, s * SLAB * JD:(s + 1) * SLAB * JD])
            ab = abp.tile([128, SLAB * G * 128], f16, tag="ab")
            nc.sync.dma_start(
                out=ab, in_=t_ablk[:, s * SLAB * G * 128:
                                   (s + 1) * SLAB * G * 128])
            ph = None
            for cc in range(SLAB):
                for g in range(G):
                    k = (s * SLAB + cc) * G + g
                    slot = k % 6
                    if slot == 0:
                        ph = psH.tile([128, 1024], fp32, tag="ph")
                    off = slot * JD if slot < 3 else 512 + (slot - 3) * JD
                    nc.tensor.matmul(
                        ph[:, off:off + JD],
                        ab[:, (cc * G + g) * 128:(cc * G + g + 1) * 128],
                        wa2[:, cc * JD:(cc + 1) * JD], start=True, stop=True)
                    if slot == 5:
                        dst = hat[:, (k - 5) * JD:(k + 1) * JD]
                        dv = dst.rearrange("p (h x) -> p h x", h=2)
                        sv = ph.rearrange("p (h x) -> p h x", h=2)[:, :, 0:480]
                        if ev[0] < DVE_EVACS:
                            nc.vector.tensor_copy(dv, sv)
                        else:
                            nc.scalar.copy(dv, sv)
                        ev[0] += 1
            pend.append((s, stage1(s, vrep0)))
            if len(pend) == 3:
                ps_, t1v_ = pend.pop(0)
                stage2(ps_, t1v_, pa, pb, True)
        while pend:
            ps_, t1v_ = pend.pop(0)
            stage2(ps_, t1v_, pa, pb, True)

        # ---------------- iter 1: v1, then pass 1 ----------------
        def sgview(t):
            return t.rearrange("p (n d) -> p n d", d=D)

        s1 = sml.tile([8, GJD], fp32, tag="s1")
        nc.scalar.copy(s1[:, 0:320], pa)
        nc.scalar.copy(s1[:, 320:640], pb)
        v1h = squash(s1, 8, G * J, "vv", f16, sgview)
        # vrep1 [128, (g, d, j)] from v1h [8, (g, j, d)]
        v1v = v1h.rearrange("p (g j d) -> p g d j", g=G, j=J)
        for half in range(2):
            pv = psV.tile([128, GJD // 2], fp32, tag="pv")
            nc.tensor.matmul(pv, s8, v1v[:, half * 2:(half + 1) * 2],
                             start=True, stop=True)
            nc.scalar.copy(vrep1[:, half * 320:(half + 1) * 320], pv)

        pa2 = psS.tile([8, GJD // 2], fp32, tag="pa")
        pb2 = psS.tile([8, GJD // 2], fp32, tag="pb")
        pend = []
        for s in range(NSLAB):
            pend.append((s, stage1(s, vrep1)))
            if len(pend) == 3:
                ps_, t1v_ = pend.pop(0)
                stage2(ps_, t1v_, pa2, pb2, False)
        while pend:
            ps_, t1v_ = pend.pop(0)
            stage2(ps_, t1v_, pa2, pb2, False)

        # ---------------- iter 2: v2 -> out ----------------
        s2 = sml.tile([8, GJD], fp32, tag="s2")
        nc.scalar.copy(s2[:, 0:320], pa2)
        nc.scalar.copy(s2[:, 320:640], pb2)
        v2 = squash(s2, 8, G * J, "vv", fp32, sgview)
        nc.sync.dma_start(out=t_out[:], in_=v2)

    nc.finalize()
    return nc


def _host_prep(x_full, W, bias):
    W = np.asarray(W, np.float32)
    wa = W.reshape(C, 16, J, E, D).transpose(1, 3, 0, 4, 2)  # [i16,e,c,d,j]
    wa = np.ascontiguousarray(wa.reshape(128, C * JD)).astype(F16)
    b2 = np.asarray(bias, np.float32).reshape(I, J)
    br = b2.reshape(C, 16, J).transpose(1, 0, 2)             # [i16,c,j]
    biasl = np.ascontiguousarray(
        np.broadcast_to(br[:, None], (16, 8, C, J)).reshape(128, C * J)
    ).astype(F16)
    m8x = np.zeros((128, 8, SN), F16)
    m8x[np.arange(128), np.arange(128) % 8, :] = 1
    m8x = m8x.reshape(128, 8 * SN)
    s8 = np.zeros((8, 128), F16)
    s8[np.arange(128) % 8, np.arange(128)] = 1
    s32 = np.zeros((32, 512), F16)
    for g in range(G):
        s32[g * 8 + np.arange(128) % 8, g * 128 + np.arange(128)] = 1
    maps = []
    for cl in range(NCORES):
        xl = np.asarray(x_full[cl * BL:(cl + 1) * BL], np.float32)
        inpT = xl.reshape(BL, C, 16, E).transpose(2, 3, 1, 0)  # [i16,e,c,b]
        inpT = np.ascontiguousarray(inpT.reshape(128, C * BL)).astype(F16)
        xr = xl.reshape(G, 8, C, 16, E)                        # [g,b8,c,i,e]
        z = np.zeros((16, 8, C, G, 16, 8), F16)
        for i in range(16):
            z[i, :, :, :, i, :] = xr[:, :, :, i, :].transpose(3, 2, 0, 1)
        ablk = z.reshape(128, C * G * 128)
        maps.append({"wa": wa, "inpT": inpT, "ablk": ablk, "biasl": biasl,
                     "m8x": m8x, "s8": s8, "s32": s32})
    return maps


_NC_CACHE = {}


def kernel(inputs, W, bias):
    from concourse import bass_utils

    if "nc" not in _NC_CACHE:
        _NC_CACHE["nc"] = _build_kernel()
    nc = _NC_CACHE["nc"]
    in_maps = _host_prep(inputs, W, bias)
    res = bass_utils.run_bass_kernel_spmd(nc, in_maps,
                                          core_ids=list(range(NCORES)))
    outs = []
    for r in res.results:
        v = r["out"].reshape(8, G, J, D).transpose(1, 0, 2, 3)  # [g,b8,j,d]
        outs.append(v.reshape(BL, J, D))
    return np.concatenate(outs, axis=0).astype(np.float32)


if __name__ == "__main__":
    import reference
    ins = reference.setup_inputs()
    ins = {k: np.asarray(v) for k, v in ins.items()}
    exp = np.asarray(reference.reference(**ins))
    got = kernel(**ins)
    err = np.abs(got - exp).max() / (np.abs(exp).max() + 1e-9)
    print("Relative error:", err)


# revision 13
# speedup vs baseline: 1.0701x; 1.0141x over previous
"""CapsuleLayer dynamic-routing kernel for 8 Trainium2 NeuronCores.

Data-parallel over batch (32 per core), W replicated. Per core:
  hat = einsum('bie,ijed->bijd') kept in SBUF f16, layout
  [p=(i%16)*8+(b%8), free=(c=i//16, g=b//8, d, j)].
  hat built by PE: stationary = host-built block-diag x matrices
  (ablk), moving = W chunks; s0 for routing iter 0 comes directly from
  inpT x W matmuls (uniform coupling).
Routing (3 iters, 2 fused passes):
  agreement  a=<hat,v>: DVE f16 mult + d-halving tree (2x mode).
  softmax    ACT exp + DVE reduce/recip.
  s = sum_i c*hat: per-(c,g,j) PE matmuls with c-selector stationaries
  (Csel[k=(i,b8), m=b8'] = c*delta), accumulated in PSUM -> no DVE mult.
"""

import sys
from contextlib import ExitStack

import numpy as np

sys.path.insert(0, "/opt/trn_rl_repo")

import ml_dtypes  # noqa: E402

F16 = ml_dtypes.float16 if hasattr(ml_dtypes, "float16") else np.float16

B, I, E = 256, 1152, 8
J, D = 10, 16
NCORES = 8
BL = B // NCORES          # 32 batches per core
C = I // 16               # 72 i-chunks of 16
G = BL // 8               # 4 b-groups of 8
JD = J * D                # 160
GJD = G * JD              # 640
CGJ = C * G * J           # 2880
FREE = C * G * JD         # 46080 free elems of hat per partition
SLAB = 9                  # c-chunks per slab
NSLAB = C // SLAB         # 8
SF = SLAB * GJD           # 5760 hat elems per slab per partition
SN = SLAB * G * J         # 360 (c,g,j) nodes per slab
NR = 3

# evac groups handled by DVE (fills pre-v0 idle window); rest go to ACT
DVE_EVACS = 2


def _build_kernel():
    import concourse.bass as bass
    import concourse.bacc as bacc
    import concourse.tile as tile
    from concourse import mybir

    fp32 = mybir.dt.float32
    f16 = mybir.dt.float16
    ADD = mybir.AluOpType.add
    MUL = mybir.AluOpType.mult

    nc = bacc.Bacc("TRN2")
    t_wa = nc.dram_tensor("wa", [128, C * JD], f16, kind="ExternalInput")
    t_inpT = nc.dram_tensor("inpT", [128, C * BL], f16, kind="ExternalInput")
    t_ablk = nc.dram_tensor("ablk", [128, C * G * 128], f16,
                            kind="ExternalInput")
    t_biasl = nc.dram_tensor("biasl", [128, C * J], f16, kind="ExternalInput")
    t_m8x = nc.dram_tensor("m8x", [128, 8 * SN], f16, kind="ExternalInput")
    t_s8 = nc.dram_tensor("s8", [8, 128], f16, kind="ExternalInput")
    t_s32 = nc.dram_tensor("s32", [32, 512], f16, kind="ExternalInput")
    t_out = nc.dram_tensor("out", [8, GJD], fp32, kind="ExternalOutput")

    def bcast(ap, pos, n):
        """Insert a broadcast (step 0, count n) free dim at free-pos pos."""
        lst = [list(x) for x in ap.ap]
        lst.insert(1 + pos, [0, n])
        return bass.AP(tensor=ap.tensor, offset=ap.offset, ap=lst)

    with ExitStack() as ctx:
        tc = ctx.enter_context(tile.TileContext(nc))
        big = ctx.enter_context(tc.tile_pool(name="big", bufs=1))
        sing = ctx.enter_context(tc.tile_pool(name="sing", bufs=1))
        wap = ctx.enter_context(tc.tile_pool(name="wap", bufs=2))
        abp = ctx.enter_context(tc.tile_pool(name="abp", bufs=2))
        p2p = ctx.enter_context(tc.tile_pool(name="p2p", bufs=2))
        trp = ctx.enter_context(tc.tile_pool(name="trp", bufs=1))
        t1p = ctx.enter_context(tc.tile_pool(name="t1p", bufs=3))
        sfp = ctx.enter_context(tc.tile_pool(name="sfp", bufs=2))
        csp = ctx.enter_context(tc.tile_pool(name="csp", bufs=2))
        # deeper pipeline lag
        sml = ctx.enter_context(tc.tile_pool(name="sml", bufs=1))
        psH = ctx.enter_context(tc.tile_pool(name="psH", bufs=2, space="PSUM"))
        ps0p = ctx.enter_context(tc.tile_pool(name="ps0p", bufs=1, space="PSUM"))
        psS = ctx.enter_context(tc.tile_pool(name="psS", bufs=1, space="PSUM"))
        psV = ctx.enter_context(tc.tile_pool(name="psV", bufs=1, space="PSUM"))

        hat = big.tile([128, FREE], f16)
        logits = sing.tile([128, CGJ], f16)
        inpT = sing.tile([128, C * BL], f16)
        biasl = sing.tile([128, C * J], f16)
        m8x = sing.tile([128, 8 * SN], f16)
        s8 = sing.tile([8, 128], f16)
        s32 = sing.tile([32, 512], f16)
        vrep0 = sing.tile([128, GJD], f16)
        vrep1 = sing.tile([128, GJD], f16)
        nc.sync.dma_start(out=inpT, in_=t_inpT[:])
        nc.sync.dma_start(out=biasl, in_=t_biasl[:])
        nc.sync.dma_start(out=m8x, in_=t_m8x[:])
        nc.sync.dma_start(out=s8, in_=t_s8[:])
        nc.sync.dma_start(out=s32, in_=t_s32[:])

        # ---------------- loop 1: s0 = (1/J) sum_i hat ----------------
        ps0 = ps0p.tile([BL, JD], fp32)
        for s in range(NSLAB):
            wa_s = wap.tile([128, SLAB * JD], f16, tag="wa")
            nc.sync.dma_start(out=wa_s,
                              in_=t_wa[:, s * SLAB * JD:(s + 1) * SLAB * JD])
            for cc in range(SLAB):
                c = s * SLAB + cc
                nc.tensor.matmul(ps0, inpT[:, c * BL:(c + 1) * BL],
                                 wa_s[:, cc * JD:(cc + 1) * JD],
                                 start=(c == 0), stop=(c == C - 1))

        # squash helpers -------------------------------------------------
        def squash(s_f32, P, nj, vname, vdt, sview):
            """v = squash(s). sview: [P, nj, 16] view builder for s-like."""
            sq = sml.tile([P, nj * D], fp32, tag=vname + "sq")
            nc.vector.tensor_mul(sq, s_f32, s_f32)
            s2 = sml.tile([P, nj], fp32, tag=vname + "s2")
            nc.vector.tensor_reduce(s2, sview(sq), axis=mybir.AxisListType.X,
                                    op=ADD)
            rt = sml.tile([P, nj], fp32, tag=vname + "rt")
            nc.scalar.sqrt(rt, s2)
            den = sml.tile([P, nj], fp32, tag=vname + "den")
            nc.vector.scalar_tensor_tensor(out=den, in0=s2, scalar=1.0,
                                           in1=rt, op0=ADD, op1=MUL)
            rden = sml.tile([P, nj], fp32, tag=vname + "rd")
            nc.vector.reciprocal(rden, den)
            sc = sml.tile([P, nj], fp32, tag=vname + "sc")
            nc.vector.tensor_mul(sc, s2, rden)
            v = sml.tile([P, nj * D], vdt, tag=vname)
            nc.vector.tensor_tensor(out=sview(v), in0=sview(s_f32),
                                    in1=bcast(sc[:, :], 1, D), op=MUL)
            return v

        # s0 is in (d, j) free order (wa column order is (d, j))
        def s0view(t):
            lst = [list(t.ap[0]), [1, J], [J, D]]
            return bass.AP(tensor=t.tensor, offset=t.offset, ap=lst)

        s0 = sml.tile([BL, JD], fp32, tag="s0")
        nc.scalar.mul(s0, ps0, 1.0 / J)
        v0h = squash(s0, BL, J, "v0", f16, s0view)

        # vrep0 [128, (g, d, j)]: vrep0[p, g] = v0h[g*8 + p%8]
        for half in range(2):
            pv = psV.tile([128, GJD // 2], fp32, tag="pv")
            for gh in range(2):
                g = half * 2 + gh
                nc.tensor.matmul(pv[:, gh * JD:(gh + 1) * JD],
                                 s32[:, g * 128:(g + 1) * 128], v0h,
                                 start=True, stop=True)
            nc.scalar.copy(vrep0[:, half * 320:(half + 1) * 320], pv)

        # ---------------- fused pass over hat ----------------
        def stage1(s, vrep):
            hs = hat[:, s * SF:(s + 1) * SF]
            p2 = p2p.tile([128, SF], f16, tag="p2")
            nc.vector.tensor_tensor(
                out=p2.rearrange("p (c f) -> p c f", c=SLAB),
                in0=hs.rearrange("p (c f) -> p c f", c=SLAB),
                in1=bcast(vrep[:, :], 0, SLAB), op=MUL)
            p2v = p2.rearrange("p (n d j) -> p n d j", d=D, j=J)
            t1 = t1p.tile([128, SN * 8], f16, tag="t1")
            t1v = t1.rearrange("p (n d j) -> p n d j", d=8, j=J)
            nc.gpsimd.tensor_tensor(out=t1v, in0=p2v[:, :, 0:8, :],
                                    in1=p2v[:, :, 8:16, :], op=ADD)
            return t1v

        def stage2(s, t1v, pa, pb, first):
            t2 = trp.tile([128, SN * 4], f16, tag="t2")
            t2v = t2.rearrange("p (n d j) -> p n d j", d=4, j=J)
            nc.vector.tensor_tensor(out=t2v, in0=t1v[:, :, 0:4, :],
                                    in1=t1v[:, :, 4:8, :], op=ADD)
            t3 = trp.tile([128, SN * 2], f16, tag="t3")
            t3v = t3.rearrange("p (n d j) -> p n d j", d=2, j=J)
            nc.vector.tensor_tensor(out=t3v, in0=t2v[:, :, 0:2, :],
                                    in1=t2v[:, :, 2:4, :], op=ADD)
            lsl = logits[:, s * SN:(s + 1) * SN]
            t4 = trp.tile([128, SN], f16, tag="t4")
            nc.vector.tensor_tensor(out=t4, in0=t3v[:, :, 0, :],
                                    in1=t3v[:, :, 1, :], op=ADD)
            if first:
                bsl = biasl[:, s * SLAB * J:(s + 1) * SLAB * J]
                nc.vector.tensor_tensor(
                    out=lsl.rearrange("p (c g j) -> p c g j", c=SLAB, g=G),
                    in0=t4.rearrange("p (c g j) -> p c g j", c=SLAB, g=G),
                    in1=bcast(bsl.rearrange("p (c j) -> p c j", c=SLAB), 1, G),
                    op=ADD)
            else:
                nc.vector.tensor_tensor(out=lsl, in0=lsl, in1=t4, op=ADD)
            ex = sfp.tile([128, SN], f16, tag="ex")
            nc.scalar.activation(ex, lsl, mybir.ActivationFunctionType.Exp)
            se = sml.tile([128, SN // J], fp32, tag="se")
            nc.vector.tensor_reduce(
                se, ex.rearrange("p (n j) -> p n j", j=J),
                axis=mybir.AxisListType.X, op=ADD)
            rse = sml.tile([128, SN // J], f16, tag="rse")
            with nc.allow_low_precision(reason="softmax denom f16"):
                nc.vector.reciprocal(rse, se)
            rsex = sfp.tile([128, SN], f16, tag="rsex")
            if first:
                nc.gpsimd.tensor_copy(rsex.rearrange("p (n j) -> p n j", j=J),
                                      bcast(rse[:, :], 1, J))
            else:
                nc.scalar.copy(rsex.rearrange("p (n j) -> p n j", j=J),
                               bcast(rse[:, :], 1, J))
            ct = sfp.tile([128, SN], f16, tag="ct")
            nc.vector.tensor_tensor(out=ct, in0=ex, in1=rsex, op=MUL)
            csel = csp.tile([128, 8 * SN], f16, tag="cs")
            nc.vector.tensor_tensor(
                out=csel.rearrange("p (col n) -> p col n", n=SN),
                in0=bcast(ct[:, :], 0, 8),
                in1=m8x.rearrange("p (col n) -> p col n", n=SN), op=MUL)
            cv = csel.rearrange("p (col n) -> p n col", col=8)
            for cc in range(SLAB):
                c = s * SLAB + cc
                for g in range(G):
                    hm = hat[:, (c * G + g) * JD:(c * G + g + 1) * JD]
                    hmv = hm.rearrange("p (d j) -> p j d", j=J)
                    dst_t = pa if g < 2 else pb
                    for j in range(J):
                        n = (cc * G + g) * J + j
                        nc.tensor.matmul(
                            dst_t[:, ((g % 2) * J + j) * D:
                                  ((g % 2) * J + j + 1) * D],
                            cv[:, n, :], hmv[:, j, :],
                            start=(c == 0), stop=(c == C - 1))

        # ---------------- loop 2: hat build + pass 0 ----------------
        pa = psS.tile([8, GJD // 2], fp32, tag="pa")
        pb = psS.tile([8, GJD // 2], fp32, tag="pb")
        ev = [0]
        for s in range(NSLAB):
            wa2 = wap.tile([128, SLAB * JD], f16, tag="wa")
            nc.sync.dma_start(out=wa2,
                              in_=t_wa[:, s * SLAB * JD:(s + 1) * SLAB * JD])
            ab = abp.tile([128, SLAB * G * 128], f16, tag="ab")
            nc.sync.dma_start(
                out=ab, in_=t_ablk[:, s * SLAB * G * 128:
                                   (s + 1) * SLAB * G * 128])
            ph = None
            for cc in range(SLAB):
                for g in range(G):
                    k = (s * SLAB + cc) * G + g
                    slot = k % 6
                    if slot == 0:
                        ph = psH.tile([128, 1024], fp32, tag="ph")
                    off = slot * JD if slot < 3 else 512 + (slot - 3) * JD
                    nc.tensor.matmul(
                        ph[:, off:off + JD],
                        ab[:, (cc * G + g) * 128:(cc * G + g + 1) * 128],
                        wa2[:, cc * JD:(cc + 1) * JD], start=True, stop=True)
                    if slot == 5:
                        dst = hat[:, (k - 5) * JD:(k + 1) * JD]
                        dv = dst.rearrange("p (h x) -> p h x", h=2)
                        sv = ph.rearrange("p (h x) -> p h x", h=2)[:, :, 0:480]
                        if ev[0] < DVE_EVACS:
                            nc.vector.tensor_copy(dv, sv)
                        else:
                            nc.scalar.copy(dv, sv)
                        ev[0] += 1
        pend = []
        for s in range(NSLAB):
            pend.append((s, stage1(s, vrep0)))
            if len(pend) == 3:
                ps_, t1v_ = pend.pop(0)
                stage2(ps_, t1v_, pa, pb, True)
        while pend:
            ps_, t1v_ = pend.pop(0)
            stage2(ps_, t1v_, pa, pb, True)

        # ---------------- iter 1: v1, then pass 1 ----------------
        def sgview(t):
            return t.rearrange("p (n d) -> p n d", d=D)

        s1 = sml.tile([8, GJD], fp32, tag="s1")
        nc.scalar.copy(s1[:, 0:320], pa)
        nc.scalar.copy(s1[:, 320:640], pb)
        v1h = squash(s1, 8, G * J, "vv", f16, sgview)
        # vrep1 [128, (g, d, j)] from v1h [8, (g, j, d)]
        v1v = v1h.rearrange("p (g j d) -> p g d j", g=G, j=J)
        for half in range(2):
            pv = psV.tile([128, GJD // 2], fp32, tag="pv")
            nc.tensor.matmul(pv, s8, v1v[:, half * 2:(half + 1) * 2],
                             start=True, stop=True)
            nc.scalar.copy(vrep1[:, half * 320:(half + 1) * 320], pv)

        pa2 = psS.tile([8, GJD // 2], fp32, tag="pa")
        pb2 = psS.tile([8, GJD // 2], fp32, tag="pb")
        pend = []
        for s in range(NSLAB):
            pend.append((s, stage1(s, vrep1)))
            if len(pend) == 3:
                ps_, t1v_ = pend.pop(0)
                stage2(ps_, t1v_, pa2, pb2, False)
        while pend:
            ps_, t1v_ = pend.pop(0)
            stage2(ps_, t1v_, pa2, pb2, False)

        # ---------------- iter 2: v2 -> out ----------------
        s2 = sml.tile([8, GJD], fp32, tag="s2")
        nc.scalar.copy(s2[:, 0:320], pa2)
        nc.scalar.copy(s2[:, 320:640], pb2)
        v2 = squash(s2, 8, G * J, "vv", fp32, sgview)
        nc.sync.dma_start(out=t_out[:], in_=v2)

    nc.finalize()
    return nc


def _host_prep(x_full, W, bias):
    W = np.asarray(W, np.float32)
    wa = W.reshape(C, 16, J, E, D).transpose(1, 3, 0, 4, 2)  # [i16,e,c,d,j]
    wa = np.ascontiguousarray(wa.reshape(128, C * JD)).astype(F16)
    b2 = np.asarray(bias, np.float32).reshape(I, J)
    br = b2.reshape(C, 16, J).transpose(1, 0, 2)             # [i16,c,j]
    biasl = np.ascontiguousarray(
        np.broadcast_to(br[:, None], (16, 8, C, J)).reshape(128, C * J)
    ).astype(F16)
    m8x = np.zeros((128, 8, SN), F16)
    m8x[np.arange(128), np.arange(128) % 8, :] = 1
    m8x = m8x.reshape(128, 8 * SN)
    s8 = np.zeros((8, 128), F16)
    s8[np.arange(128) % 8, np.arange(128)] = 1
    s32 = np.zeros((32, 512), F16)
    for g in range(G):
        s32[g * 8 + np.arange(128) % 8, g * 128 + np.arange(128)] = 1
    maps = []
    for cl in range(NCORES):
        xl = np.asarray(x_full[cl * BL:(cl + 1) * BL], np.float32)
        inpT = xl.reshape(BL, C, 16, E).transpose(2, 3, 1, 0)  # [i16,e,c,b]
        inpT = np.ascontiguousarray(inpT.reshape(128, C * BL)).astype(F16)
        xr = xl.reshape(G, 8, C, 16, E)                        # [g,b8,c,i,e]
        z = np.zeros((16, 8, C, G, 16, 8), F16)
        for i in range(16):
            z[i, :, :, :, i, :] = xr[:, :, :, i, :].transpose(3, 2, 0, 1)
        ablk = z.reshape(128, C * G * 128)
        maps.append({"wa": wa, "inpT": inpT, "ablk": ablk, "biasl": biasl,
                     "m8x": m8x, "s8": s8, "s32": s32})
    return maps


_NC_CACHE = {}


def kernel(inputs, W, bias):
    from concourse import bass_utils

    if "nc" not in _NC_CACHE:
        _NC_CACHE["nc"] = _build_kernel()
    nc = _NC_CACHE["nc"]
    in_maps = _host_prep(inputs, W, bias)
    res = bass_utils.run_bass_kernel_spmd(nc, in_maps,
                                          core_ids=list(range(NCORES)))
    outs = []
    for r in res.results:
        v = r["out"].reshape(8, G, J, D).transpose(1, 0, 2, 3)  # [g,b8,j,d]
        outs.append(v.reshape(BL, J, D))
    return np.concatenate(outs, axis=0).astype(np.float32)


if __name__ == "__main__":
    import reference
    ins = reference.setup_inputs()
    ins = {k: np.asarray(v) for k, v in ins.items()}
    exp = np.asarray(reference.reference(**ins))
    got = kernel(**ins)
    err = np.abs(got - exp).max() / (np.abs(exp).max() + 1e-9)
    print("Relative error:", err)


# revision 14
# speedup vs baseline: 1.0794x; 1.0087x over previous
"""CapsuleLayer dynamic-routing kernel for 8 Trainium2 NeuronCores.

Data-parallel over batch (32 per core), W replicated. Per core:
  hat = einsum('bie,ijed->bijd') kept in SBUF f16, layout
  [p=(i%16)*8+(b%8), free=(c=i//16, g=b//8, d, j)].
  hat built by PE: stationary = host-built block-diag x matrices
  (ablk), moving = W chunks; s0 for routing iter 0 comes directly from
  inpT x W matmuls (uniform coupling).
Routing (3 iters, 2 fused passes):
  agreement  a=<hat,v>: DVE f16 mult + d-halving tree (2x mode).
  softmax    ACT exp + DVE reduce/recip.
  s = sum_i c*hat: per-(c,g,j) PE matmuls with c-selector stationaries
  (Csel[k=(i,b8), m=b8'] = c*delta), accumulated in PSUM -> no DVE mult.
"""

import sys
from contextlib import ExitStack

import numpy as np

sys.path.insert(0, "/opt/trn_rl_repo")

import ml_dtypes  # noqa: E402

F16 = ml_dtypes.float16 if hasattr(ml_dtypes, "float16") else np.float16

B, I, E = 256, 1152, 8
J, D = 10, 16
NCORES = 8
BL = B // NCORES          # 32 batches per core
C = I // 16               # 72 i-chunks of 16
G = BL // 8               # 4 b-groups of 8
JD = J * D                # 160
GJD = G * JD              # 640
CGJ = C * G * J           # 2880
FREE = C * G * JD         # 46080 free elems of hat per partition
SLAB = 9                  # c-chunks per slab
NSLAB = C // SLAB         # 8
SF = SLAB * GJD           # 5760 hat elems per slab per partition
SN = SLAB * G * J         # 360 (c,g,j) nodes per slab
NR = 3

# evac groups handled by DVE (fills pre-v0 idle window); rest go to ACT
DVE_EVACS = 2


def _build_kernel():
    import concourse.bass as bass
    import concourse.bacc as bacc
    import concourse.tile as tile
    from concourse import mybir

    fp32 = mybir.dt.float32
    f16 = mybir.dt.float16
    ADD = mybir.AluOpType.add
    MUL = mybir.AluOpType.mult

    nc = bacc.Bacc("TRN2")
    t_wa = nc.dram_tensor("wa", [128, C * JD], f16, kind="ExternalInput")
    t_inpT = nc.dram_tensor("inpT", [128, C * BL], f16, kind="ExternalInput")
    t_ablk = nc.dram_tensor("ablk", [128, C * G * 128], f16,
                            kind="ExternalInput")
    t_biasl = nc.dram_tensor("biasl", [128, C * J], f16, kind="ExternalInput")
    t_m8x = nc.dram_tensor("m8x", [128, 8 * SN], f16, kind="ExternalInput")
    t_s8 = nc.dram_tensor("s8", [8, 128], f16, kind="ExternalInput")
    t_s32 = nc.dram_tensor("s32", [32, 512], f16, kind="ExternalInput")
    t_out = nc.dram_tensor("out", [8, GJD], fp32, kind="ExternalOutput")

    def bcast(ap, pos, n):
        """Insert a broadcast (step 0, count n) free dim at free-pos pos."""
        lst = [list(x) for x in ap.ap]
        lst.insert(1 + pos, [0, n])
        return bass.AP(tensor=ap.tensor, offset=ap.offset, ap=lst)

    with ExitStack() as ctx:
        tc = ctx.enter_context(tile.TileContext(nc))
        big = ctx.enter_context(tc.tile_pool(name="big", bufs=1))
        sing = ctx.enter_context(tc.tile_pool(name="sing", bufs=1))
        wap = ctx.enter_context(tc.tile_pool(name="wap", bufs=2))
        abp = ctx.enter_context(tc.tile_pool(name="abp", bufs=3))
        p2p = ctx.enter_context(tc.tile_pool(name="p2p", bufs=3))
        trp = ctx.enter_context(tc.tile_pool(name="trp", bufs=1))
        t1p = ctx.enter_context(tc.tile_pool(name="t1p", bufs=2))
        sfp = ctx.enter_context(tc.tile_pool(name="sfp", bufs=1))
        csp = ctx.enter_context(tc.tile_pool(name="csp", bufs=2))
        # deeper pipeline lag
        sml = ctx.enter_context(tc.tile_pool(name="sml", bufs=1))
        psH = ctx.enter_context(tc.tile_pool(name="psH", bufs=2, space="PSUM"))
        ps0p = ctx.enter_context(tc.tile_pool(name="ps0p", bufs=1, space="PSUM"))
        psS = ctx.enter_context(tc.tile_pool(name="psS", bufs=1, space="PSUM"))
        psV = ctx.enter_context(tc.tile_pool(name="psV", bufs=1, space="PSUM"))

        hat = big.tile([128, FREE], f16)
        logits = sing.tile([128, CGJ], f16)
        inpT = sing.tile([128, C * BL], f16)
        biasl = sing.tile([128, C * J], f16)
        m8x = sing.tile([128, 8 * SN], f16)
        s8 = sing.tile([8, 128], f16)
        s32 = sing.tile([32, 512], f16)
        vrep0 = sing.tile([128, GJD], f16)
        vrep1 = sing.tile([128, GJD], f16)
        nc.sync.dma_start(out=inpT, in_=t_inpT[:])
        nc.sync.dma_start(out=biasl, in_=t_biasl[:])
        nc.sync.dma_start(out=m8x, in_=t_m8x[:])
        nc.sync.dma_start(out=s8, in_=t_s8[:])
        nc.sync.dma_start(out=s32, in_=t_s32[:])

        # ---------------- loop 1: s0 = (1/J) sum_i hat ----------------
        ps0 = ps0p.tile([BL, JD], fp32)
        for s in range(NSLAB):
            wa_s = wap.tile([128, SLAB * JD], f16, tag="wa")
            nc.sync.dma_start(out=wa_s,
                              in_=t_wa[:, s * SLAB * JD:(s + 1) * SLAB * JD])
            for cc in range(SLAB):
                c = s * SLAB + cc
                nc.tensor.matmul(ps0, inpT[:, c * BL:(c + 1) * BL],
                                 wa_s[:, cc * JD:(cc + 1) * JD],
                                 start=(c == 0), stop=(c == C - 1))

        # squash helpers -------------------------------------------------
        def squash(s_f32, P, nj, vname, vdt, sview):
            """v = squash(s). sview: [P, nj, 16] view builder for s-like."""
            sq = sml.tile([P, nj * D], fp32, tag=vname + "sq")
            nc.vector.tensor_mul(sq, s_f32, s_f32)
            s2 = sml.tile([P, nj], fp32, tag=vname + "s2")
            nc.vector.tensor_reduce(s2, sview(sq), axis=mybir.AxisListType.X,
                                    op=ADD)
            rt = sml.tile([P, nj], fp32, tag=vname + "rt")
            nc.scalar.sqrt(rt, s2)
            den = sml.tile([P, nj], fp32, tag=vname + "den")
            nc.vector.scalar_tensor_tensor(out=den, in0=s2, scalar=1.0,
                                           in1=rt, op0=ADD, op1=MUL)
            rden = sml.tile([P, nj], fp32, tag=vname + "rd")
            nc.vector.reciprocal(rden, den)
            sc = sml.tile([P, nj], fp32, tag=vname + "sc")
            nc.vector.tensor_mul(sc, s2, rden)
            v = sml.tile([P, nj * D], vdt, tag=vname)
            nc.vector.tensor_tensor(out=sview(v), in0=sview(s_f32),
                                    in1=bcast(sc[:, :], 1, D), op=MUL)
            return v

        # s0 is in (d, j) free order (wa column order is (d, j))
        def s0view(t):
            lst = [list(t.ap[0]), [1, J], [J, D]]
            return bass.AP(tensor=t.tensor, offset=t.offset, ap=lst)

        s0 = sml.tile([BL, JD], fp32, tag="s0")
        nc.scalar.mul(s0, ps0, 1.0 / J)
        v0h = squash(s0, BL, J, "v0", f16, s0view)

        # vrep0 [128, (g, d, j)]: vrep0[p, g] = v0h[g*8 + p%8]
        for half in range(2):
            pv = psV.tile([128, GJD // 2], fp32, tag="pv")
            for gh in range(2):
                g = half * 2 + gh
                nc.tensor.matmul(pv[:, gh * JD:(gh + 1) * JD],
                                 s32[:, g * 128:(g + 1) * 128], v0h,
                                 start=True, stop=True)
            nc.scalar.copy(vrep0[:, half * 320:(half + 1) * 320], pv)

        # ---------------- fused pass over hat ----------------
        def stage1(s, vrep):
            hs = hat[:, s * SF:(s + 1) * SF]
            p2 = p2p.tile([128, SF], f16, tag="p2")
            nc.vector.tensor_tensor(
                out=p2.rearrange("p (c f) -> p c f", c=SLAB),
                in0=hs.rearrange("p (c f) -> p c f", c=SLAB),
                in1=bcast(vrep[:, :], 0, SLAB), op=MUL)
            p2v = p2.rearrange("p (n d j) -> p n d j", d=D, j=J)
            t1 = t1p.tile([128, SN * 8], f16, tag="t1")
            t1v = t1.rearrange("p (n d j) -> p n d j", d=8, j=J)
            nc.gpsimd.tensor_tensor(out=t1v, in0=p2v[:, :, 0:8, :],
                                    in1=p2v[:, :, 8:16, :], op=ADD)
            return t1v

        def stage2(s, t1v, pa, pb, first):
            t2 = trp.tile([128, SN * 4], f16, tag="t2")
            t2v = t2.rearrange("p (n d j) -> p n d j", d=4, j=J)
            nc.vector.tensor_tensor(out=t2v, in0=t1v[:, :, 0:4, :],
                                    in1=t1v[:, :, 4:8, :], op=ADD)
            t3 = trp.tile([128, SN * 2], f16, tag="t3")
            t3v = t3.rearrange("p (n d j) -> p n d j", d=2, j=J)
            nc.vector.tensor_tensor(out=t3v, in0=t2v[:, :, 0:2, :],
                                    in1=t2v[:, :, 2:4, :], op=ADD)
            lsl = logits[:, s * SN:(s + 1) * SN]
            t4 = trp.tile([128, SN], f16, tag="t4")
            nc.vector.tensor_tensor(out=t4, in0=t3v[:, :, 0, :],
                                    in1=t3v[:, :, 1, :], op=ADD)
            if first:
                bsl = biasl[:, s * SLAB * J:(s + 1) * SLAB * J]
                nc.vector.tensor_tensor(
                    out=lsl.rearrange("p (c g j) -> p c g j", c=SLAB, g=G),
                    in0=t4.rearrange("p (c g j) -> p c g j", c=SLAB, g=G),
                    in1=bcast(bsl.rearrange("p (c j) -> p c j", c=SLAB), 1, G),
                    op=ADD)
            else:
                nc.vector.tensor_tensor(out=lsl, in0=lsl, in1=t4, op=ADD)
            ex = sfp.tile([128, SN], f16, tag="ex")
            nc.scalar.activation(ex, lsl, mybir.ActivationFunctionType.Exp)
            se = sml.tile([128, SN // J], fp32, tag="se")
            nc.vector.tensor_reduce(
                se, ex.rearrange("p (n j) -> p n j", j=J),
                axis=mybir.AxisListType.X, op=ADD)
            rse = sml.tile([128, SN // J], f16, tag="rse")
            with nc.allow_low_precision(reason="softmax denom f16"):
                nc.vector.reciprocal(rse, se)
            rsex = sfp.tile([128, SN], f16, tag="rsex")
            if first:
                nc.gpsimd.tensor_copy(rsex.rearrange("p (n j) -> p n j", j=J),
                                      bcast(rse[:, :], 1, J))
            else:
                nc.scalar.copy(rsex.rearrange("p (n j) -> p n j", j=J),
                               bcast(rse[:, :], 1, J))
            ct = sfp.tile([128, SN], f16, tag="ct")
            nc.vector.tensor_tensor(out=ct, in0=ex, in1=rsex, op=MUL)
            csel = csp.tile([128, 8 * SN], f16, tag="cs")
            nc.vector.tensor_tensor(
                out=csel.rearrange("p (col n) -> p col n", n=SN),
                in0=bcast(ct[:, :], 0, 8),
                in1=m8x.rearrange("p (col n) -> p col n", n=SN), op=MUL)
            cv = csel.rearrange("p (col n) -> p n col", col=8)
            for cc in range(SLAB):
                c = s * SLAB + cc
                for g in range(G):
                    hm = hat[:, (c * G + g) * JD:(c * G + g + 1) * JD]
                    hmv = hm.rearrange("p (d j) -> p j d", j=J)
                    dst_t = pa if g < 2 else pb
                    for j in range(J):
                        n = (cc * G + g) * J + j
                        nc.tensor.matmul(
                            dst_t[:, ((g % 2) * J + j) * D:
                                  ((g % 2) * J + j + 1) * D],
                            cv[:, n, :], hmv[:, j, :],
                            start=(c == 0), stop=(c == C - 1))

        # ---------------- loop 2: hat build + pass 0 ----------------
        pa = psS.tile([8, GJD // 2], fp32, tag="pa")
        pb = psS.tile([8, GJD // 2], fp32, tag="pb")
        ev = [0]
        for s in range(NSLAB):
            wa2 = wap.tile([128, SLAB * JD], f16, tag="wa")
            nc.sync.dma_start(out=wa2,
                              in_=t_wa[:, s * SLAB * JD:(s + 1) * SLAB * JD])
            ab = None
            ph = None
            for cc in range(SLAB):
                for g in range(G):
                    kk = cc * G + g              # slab-local (c,g) index
                    k = (s * SLAB + cc) * G + g
                    if kk % 18 == 0:
                        ab = abp.tile([128, 18 * 128], f16, tag="ab")
                        base = s * SLAB * G * 128 + kk * 128
                        nc.sync.dma_start(
                            out=ab, in_=t_ablk[:, base:base + 18 * 128])
                    slot = k % 6
                    if slot == 0:
                        ph = psH.tile([128, 1024], fp32, tag="ph")
                    off = slot * JD if slot < 3 else 512 + (slot - 3) * JD
                    nc.tensor.matmul(
                        ph[:, off:off + JD],
                        ab[:, (kk % 18) * 128:(kk % 18 + 1) * 128],
                        wa2[:, cc * JD:(cc + 1) * JD], start=True, stop=True)
                    if slot == 5:
                        dst = hat[:, (k - 5) * JD:(k + 1) * JD]
                        dv = dst.rearrange("p (h x) -> p h x", h=2)
                        sv = ph.rearrange("p (h x) -> p h x", h=2)[:, :, 0:480]
                        if ev[0] < DVE_EVACS:
                            nc.vector.tensor_copy(dv, sv)
                        else:
                            nc.scalar.copy(dv, sv)
                        ev[0] += 1
        pend = []
        for s in range(NSLAB):
            pend.append((s, stage1(s, vrep0)))
            if len(pend) == 3:
                ps_, t1v_ = pend.pop(0)
                stage2(ps_, t1v_, pa, pb, True)
        while pend:
            ps_, t1v_ = pend.pop(0)
            stage2(ps_, t1v_, pa, pb, True)

        # ---------------- iter 1: v1, then pass 1 ----------------
        def sgview(t):
            return t.rearrange("p (n d) -> p n d", d=D)

        s1 = sml.tile([8, GJD], fp32, tag="s1")
        nc.scalar.copy(s1[:, 0:320], pa)
        nc.scalar.copy(s1[:, 320:640], pb)
        v1h = squash(s1, 8, G * J, "vv", f16, sgview)
        # vrep1 [128, (g, d, j)] from v1h [8, (g, j, d)]
        v1v = v1h.rearrange("p (g j d) -> p g d j", g=G, j=J)
        for half in range(2):
            pv = psV.tile([128, GJD // 2], fp32, tag="pv")
            nc.tensor.matmul(pv, s8, v1v[:, half * 2:(half + 1) * 2],
                             start=True, stop=True)
            nc.scalar.copy(vrep1[:, half * 320:(half + 1) * 320], pv)

        pa2 = psS.tile([8, GJD // 2], fp32, tag="pa")
        pb2 = psS.tile([8, GJD // 2], fp32, tag="pb")
        pend = []
        for s in range(NSLAB):
            pend.append((s, stage1(s, vrep1)))
            if len(pend) == 3:
                ps_, t1v_ = pend.pop(0)
                stage2(ps_, t1v_, pa2, pb2, False)
        while pend:
            ps_, t1v_ = pend.pop(0)
            stage2(ps_, t1v_, pa2, pb2, False)

        # ---------------- iter 2: v2 -> out ----------------
        s2 = sml.tile([8, GJD], fp32, tag="s2")
        nc.scalar.copy(s2[:, 0:320], pa2)
        nc.scalar.copy(s2[:, 320:640], pb2)
        v2 = squash(s2, 8, G * J, "vv", fp32, sgview)
        nc.sync.dma_start(out=t_out[:], in_=v2)

    nc.finalize()
    return nc


def _host_prep(x_full, W, bias):
    W = np.asarray(W, np.float32)
    wa = W.reshape(C, 16, J, E, D).transpose(1, 3, 0, 4, 2)  # [i16,e,c,d,j]
    wa = np.ascontiguousarray(wa.reshape(128, C * JD)).astype(F16)
    b2 = np.asarray(bias, np.float32).reshape(I, J)
    br = b2.reshape(C, 16, J).transpose(1, 0, 2)             # [i16,c,j]
    biasl = np.ascontiguousarray(
        np.broadcast_to(br[:, None], (16, 8, C, J)).reshape(128, C * J)
    ).astype(F16)
    m8x = np.zeros((128, 8, SN), F16)
    m8x[np.arange(128), np.arange(128) % 8, :] = 1
    m8x = m8x.reshape(128, 8 * SN)
    s8 = np.zeros((8, 128), F16)
    s8[np.arange(128) % 8, np.arange(128)] = 1
    s32 = np.zeros((32, 512), F16)
    for g in range(G):
        s32[g * 8 + np.arange(128) % 8, g * 128 + np.arange(128)] = 1
    maps = []
    for cl in range(NCORES):
        xl = np.asarray(x_full[cl * BL:(cl + 1) * BL], np.float32)
        inpT = xl.reshape(BL, C, 16, E).transpose(2, 3, 1, 0)  # [i16,e,c,b]
        inpT = np.ascontiguousarray(inpT.reshape(128, C * BL)).astype(F16)
        xr = xl.reshape(G, 8, C, 16, E)                        # [g,b8,c,i,e]
        z = np.zeros((16, 8, C, G, 16, 8), F16)
        for i in range(16):
            z[i, :, :, :, i, :] = xr[:, :, :, i, :].transpose(3, 2, 0, 1)
        ablk = z.reshape(128, C * G * 128)
        maps.append({"wa": wa, "inpT": inpT, "ablk": ablk, "biasl": biasl,
                     "m8x": m8x, "s8": s8, "s32": s32})
    return maps


_NC_CACHE = {}


def kernel(inputs, W, bias):
    from concourse import bass_utils

    if "nc" not in _NC_CACHE:
        _NC_CACHE["nc"] = _build_kernel()
    nc = _NC_CACHE["nc"]
    in_maps = _host_prep(inputs, W, bias)
    res = bass_utils.run_bass_kernel_spmd(nc, in_maps,
                                          core_ids=list(range(NCORES)))
    outs = []
    for r in res.results:
        v = r["out"].reshape(8, G, J, D).transpose(1, 0, 2, 3)  # [g,b8,j,d]
        outs.append(v.reshape(BL, J, D))
    return np.concatenate(outs, axis=0).astype(np.float32)


if __name__ == "__main__":
    import reference
    ins = reference.setup_inputs()
    ins = {k: np.asarray(v) for k, v in ins.items()}
    exp = np.asarray(reference.reference(**ins))
    got = kernel(**ins)
    err = np.abs(got - exp).max() / (np.abs(exp).max() + 1e-9)
    print("Relative error:", err)


# revision 15
# speedup vs baseline: 1.1559x; 1.0709x over previous
"""CapsuleLayer dynamic-routing kernel for 8 Trainium2 NeuronCores.

Data-parallel over batch (32 per core), W replicated. Per core:
  hat = einsum('bie,ijed->bijd') kept in SBUF f16, layout
  [p=(i%16)*8+(b%8), free=(c=i//16, g=b//8, d, j)].
  hat built by PE: stationary = host-built block-diag x matrices
  (ablk), moving = W chunks; s0 for routing iter 0 comes directly from
  inpT x W matmuls (uniform coupling).
Routing (3 iters, 2 fused passes):
  agreement  a=<hat,v>: DVE f16 mult + d-halving tree (2x mode).
  softmax    ACT exp + DVE reduce/recip.
  s = sum_i c*hat: per-(c,g,j) PE matmuls with c-selector stationaries
  (Csel[k=(i,b8), m=b8'] = c*delta), accumulated in PSUM -> no DVE mult.
"""

import sys
from contextlib import ExitStack

import numpy as np

sys.path.insert(0, "/opt/trn_rl_repo")

import ml_dtypes  # noqa: E402

F16 = ml_dtypes.float16 if hasattr(ml_dtypes, "float16") else np.float16

B, I, E = 256, 1152, 8
J, D = 10, 16
NCORES = 8
BL = B // NCORES          # 32 batches per core
C = I // 16               # 72 i-chunks of 16
G = BL // 8               # 4 b-groups of 8
JD = J * D                # 160
GJD = G * JD              # 640
CGJ = C * G * J           # 2880
FREE = C * G * JD         # 46080 free elems of hat per partition
SLAB = 9                  # c-chunks per slab
NSLAB = C // SLAB         # 8
SF = SLAB * GJD           # 5760 hat elems per slab per partition
SN = SLAB * G * J         # 360 (c,g,j) nodes per slab
NR = 3

# evac groups handled by DVE (fills pre-v0 idle window); rest go to ACT
DVE_EVACS = 2


def _build_kernel():
    import concourse.bass as bass
    import concourse.bacc as bacc
    import concourse.tile as tile
    from concourse import mybir

    fp32 = mybir.dt.float32
    f16 = mybir.dt.float16
    ADD = mybir.AluOpType.add
    MUL = mybir.AluOpType.mult

    nc = bacc.Bacc("TRN2")
    t_wa = nc.dram_tensor("wa", [128, C * JD], f16, kind="ExternalInput")
    t_inpT = nc.dram_tensor("inpT", [128, C * BL], f16, kind="ExternalInput")
    t_ablk = nc.dram_tensor("ablk", [128, C * G * 128], f16,
                            kind="ExternalInput")
    t_biasl = nc.dram_tensor("biasl", [128, C * J], f16, kind="ExternalInput")
    t_m8x = nc.dram_tensor("m8x", [128, 8 * SN], f16, kind="ExternalInput")
    t_s8 = nc.dram_tensor("s8", [8, 128], f16, kind="ExternalInput")
    t_s32 = nc.dram_tensor("s32", [32, 512], f16, kind="ExternalInput")
    t_out = nc.dram_tensor("out", [8, GJD], fp32, kind="ExternalOutput")

    def bcast(ap, pos, n):
        """Insert a broadcast (step 0, count n) free dim at free-pos pos."""
        lst = [list(x) for x in ap.ap]
        lst.insert(1 + pos, [0, n])
        return bass.AP(tensor=ap.tensor, offset=ap.offset, ap=lst)

    with ExitStack() as ctx:
        tc = ctx.enter_context(tile.TileContext(nc))
        big = ctx.enter_context(tc.tile_pool(name="big", bufs=1))
        sing = ctx.enter_context(tc.tile_pool(name="sing", bufs=1))
        wap = ctx.enter_context(tc.tile_pool(name="wap", bufs=2))
        abp = ctx.enter_context(tc.tile_pool(name="abp", bufs=3))
        p2p = ctx.enter_context(tc.tile_pool(name="p2p", bufs=3))
        trp = ctx.enter_context(tc.tile_pool(name="trp", bufs=1))
        t1p = ctx.enter_context(tc.tile_pool(name="t1p", bufs=2))
        sfp = ctx.enter_context(tc.tile_pool(name="sfp", bufs=1))
        csp = ctx.enter_context(tc.tile_pool(name="csp", bufs=2))
        # deeper pipeline lag
        sml = ctx.enter_context(tc.tile_pool(name="sml", bufs=1))
        psH = ctx.enter_context(tc.tile_pool(name="psH", bufs=2, space="PSUM"))
        ps0p = ctx.enter_context(tc.tile_pool(name="ps0p", bufs=1, space="PSUM"))
        psS = ctx.enter_context(tc.tile_pool(name="psS", bufs=1, space="PSUM"))
        psV = ctx.enter_context(tc.tile_pool(name="psV", bufs=1, space="PSUM"))

        hat = big.tile([128, FREE], f16)
        logits = sing.tile([128, CGJ], f16)
        inpT = sing.tile([128, C * BL], f16)
        biasl = sing.tile([128, C * J], f16)
        m8x = sing.tile([128, 8 * SN], f16)
        s8 = sing.tile([8, 128], f16)
        s32 = sing.tile([32, 512], f16)
        vrep0 = sing.tile([128, GJD], f16)
        vrep1 = sing.tile([128, GJD], f16)
        nc.sync.dma_start(out=inpT, in_=t_inpT[:])
        nc.sync.dma_start(out=biasl, in_=t_biasl[:])
        nc.sync.dma_start(out=m8x, in_=t_m8x[:])
        nc.sync.dma_start(out=s8, in_=t_s8[:])
        nc.sync.dma_start(out=s32, in_=t_s32[:])

        # ---------------- loop 1: s0 = (1/J) sum_i hat ----------------
        ps0 = ps0p.tile([BL, JD], fp32)
        for s in range(NSLAB):
            wa_s = wap.tile([128, SLAB * JD], f16, tag="wa")
            nc.sync.dma_start(out=wa_s,
                              in_=t_wa[:, s * SLAB * JD:(s + 1) * SLAB * JD])
            for cc in range(SLAB):
                c = s * SLAB + cc
                nc.tensor.matmul(ps0, inpT[:, c * BL:(c + 1) * BL],
                                 wa_s[:, cc * JD:(cc + 1) * JD],
                                 start=(c == 0), stop=(c == C - 1))

        # squash helpers -------------------------------------------------
        def squash(s_f32, P, nj, vname, vdt, sview):
            """v = squash(s). sview: [P, nj, 16] view builder for s-like."""
            sq = sml.tile([P, nj * D], fp32, tag=vname + "sq")
            nc.vector.tensor_mul(sq, s_f32, s_f32)
            s2 = sml.tile([P, nj], fp32, tag=vname + "s2")
            nc.vector.tensor_reduce(s2, sview(sq), axis=mybir.AxisListType.X,
                                    op=ADD)
            rt = sml.tile([P, nj], fp32, tag=vname + "rt")
            nc.scalar.sqrt(rt, s2)
            den = sml.tile([P, nj], fp32, tag=vname + "den")
            nc.vector.scalar_tensor_tensor(out=den, in0=s2, scalar=1.0,
                                           in1=rt, op0=ADD, op1=MUL)
            rden = sml.tile([P, nj], fp32, tag=vname + "rd")
            nc.vector.reciprocal(rden, den)
            sc = sml.tile([P, nj], fp32, tag=vname + "sc")
            nc.vector.tensor_mul(sc, s2, rden)
            v = sml.tile([P, nj * D], vdt, tag=vname)
            nc.vector.tensor_tensor(out=sview(v), in0=sview(s_f32),
                                    in1=bcast(sc[:, :], 1, D), op=MUL)
            return v

        # s0 is in (d, j) free order (wa column order is (d, j))
        def s0view(t):
            lst = [list(t.ap[0]), [1, J], [J, D]]
            return bass.AP(tensor=t.tensor, offset=t.offset, ap=lst)

        s0 = sml.tile([BL, JD], fp32, tag="s0")
        nc.scalar.mul(s0, ps0, 1.0 / J)
        v0h = squash(s0, BL, J, "v0", f16, s0view)

        # vrep0 [128, (g, d, j)]: vrep0[p, g] = v0h[g*8 + p%8]
        for half in range(2):
            pv = psV.tile([128, GJD // 2], fp32, tag="pv")
            for gh in range(2):
                g = half * 2 + gh
                nc.tensor.matmul(pv[:, gh * JD:(gh + 1) * JD],
                                 s32[:, g * 128:(g + 1) * 128], v0h,
                                 start=True, stop=True)
            nc.scalar.copy(vrep0[:, half * 320:(half + 1) * 320], pv)

        # ---------------- fused pass over hat ----------------
        def stage1(s, vrep):
            hs = hat[:, s * SF:(s + 1) * SF]
            p2 = p2p.tile([128, SF], f16, tag="p2")
            nc.vector.tensor_tensor(
                out=p2.rearrange("p (c f) -> p c f", c=SLAB),
                in0=hs.rearrange("p (c f) -> p c f", c=SLAB),
                in1=bcast(vrep[:, :], 0, SLAB), op=MUL)
            p2v = p2.rearrange("p (n d j) -> p n d j", d=D, j=J)
            t1 = t1p.tile([128, SN * 8], f16, tag="t1")
            t1v = t1.rearrange("p (n d j) -> p n d j", d=8, j=J)
            nc.gpsimd.tensor_tensor(out=t1v, in0=p2v[:, :, 0:8, :],
                                    in1=p2v[:, :, 8:16, :], op=ADD)
            return t1v

        def stage2(s, t1v, pa, pb, first):
            t2 = trp.tile([128, SN * 4], f16, tag="t2")
            t2v = t2.rearrange("p (n d j) -> p n d j", d=4, j=J)
            nc.vector.tensor_tensor(out=t2v, in0=t1v[:, :, 0:4, :],
                                    in1=t1v[:, :, 4:8, :], op=ADD)
            t3 = trp.tile([128, SN * 2], f16, tag="t3")
            t3v = t3.rearrange("p (n d j) -> p n d j", d=2, j=J)
            nc.vector.tensor_tensor(out=t3v, in0=t2v[:, :, 0:2, :],
                                    in1=t2v[:, :, 2:4, :], op=ADD)
            lsl = logits[:, s * SN:(s + 1) * SN]
            t4 = trp.tile([128, SN], f16, tag="t4")
            nc.vector.tensor_tensor(out=t4, in0=t3v[:, :, 0, :],
                                    in1=t3v[:, :, 1, :], op=ADD)
            if first:
                bsl = biasl[:, s * SLAB * J:(s + 1) * SLAB * J]
                nc.vector.tensor_tensor(
                    out=lsl.rearrange("p (c g j) -> p c g j", c=SLAB, g=G),
                    in0=t4.rearrange("p (c g j) -> p c g j", c=SLAB, g=G),
                    in1=bcast(bsl.rearrange("p (c j) -> p c j", c=SLAB), 1, G),
                    op=ADD)
            else:
                nc.vector.tensor_tensor(out=lsl, in0=lsl, in1=t4, op=ADD)
            ex = sfp.tile([128, SN], f16, tag="ex")
            nc.scalar.activation(ex, lsl, mybir.ActivationFunctionType.Exp)
            se = sml.tile([128, SN // J], fp32, tag="se")
            nc.vector.tensor_reduce(
                se, ex.rearrange("p (n j) -> p n j", j=J),
                axis=mybir.AxisListType.X, op=ADD)
            rse = sml.tile([128, SN // J], f16, tag="rse")
            with nc.allow_low_precision(reason="softmax denom f16"):
                nc.vector.reciprocal(rse, se)
            rsex = sfp.tile([128, SN], f16, tag="rsex")
            nc.vector.tensor_copy(rsex.rearrange("p (n j) -> p n j", j=J),
                                  bcast(rse[:, :], 1, J))
            ct = sfp.tile([128, SN], f16, tag="ct")
            nc.vector.tensor_tensor(out=ct, in0=ex, in1=rsex, op=MUL)
            csel = csp.tile([128, 8 * SN], f16, tag="cs")
            nc.vector.tensor_tensor(
                out=csel.rearrange("p (col n) -> p col n", n=SN),
                in0=bcast(ct[:, :], 0, 8),
                in1=m8x.rearrange("p (col n) -> p col n", n=SN), op=MUL)
            cv = csel.rearrange("p (col n) -> p n col", col=8)
            for cc in range(SLAB):
                c = s * SLAB + cc
                for g in range(G):
                    hm = hat[:, (c * G + g) * JD:(c * G + g + 1) * JD]
                    hmv = hm.rearrange("p (d j) -> p j d", j=J)
                    dst_t = pa if g < 2 else pb
                    for j in range(J):
                        n = (cc * G + g) * J + j
                        nc.tensor.matmul(
                            dst_t[:, ((g % 2) * J + j) * D:
                                  ((g % 2) * J + j + 1) * D],
                            cv[:, n, :], hmv[:, j, :],
                            start=(c == 0), stop=(c == C - 1))

        # ---------------- loop 2: hat build + pass 0 ----------------
        pa = psS.tile([8, GJD // 2], fp32, tag="pa")
        pb = psS.tile([8, GJD // 2], fp32, tag="pb")
        ev = [0]
        pend = []
        for s in range(NSLAB):
            wa2 = wap.tile([128, SLAB * JD], f16, tag="wa")
            nc.sync.dma_start(out=wa2,
                              in_=t_wa[:, s * SLAB * JD:(s + 1) * SLAB * JD])
            ab = None
            ph = None
            for cc in range(SLAB):
                for g in range(G):
                    kk = cc * G + g              # slab-local (c,g) index
                    k = (s * SLAB + cc) * G + g
                    if kk % 18 == 0:
                        ab = abp.tile([128, 18 * 128], f16, tag="ab")
                        base = s * SLAB * G * 128 + kk * 128
                        nc.sync.dma_start(
                            out=ab, in_=t_ablk[:, base:base + 18 * 128])
                    slot = k % 6
                    if slot == 0:
                        ph = psH.tile([128, 1024], fp32, tag="ph")
                    off = slot * JD if slot < 3 else 512 + (slot - 3) * JD
                    nc.tensor.matmul(
                        ph[:, off:off + JD],
                        ab[:, (kk % 18) * 128:(kk % 18 + 1) * 128],
                        wa2[:, cc * JD:(cc + 1) * JD], start=True, stop=True)
                    if slot == 5:
                        dst = hat[:, (k - 5) * JD:(k + 1) * JD]
                        dv = dst.rearrange("p (h x) -> p h x", h=2)
                        sv = ph.rearrange("p (h x) -> p h x", h=2)[:, :, 0:480]
                        if ev[0] < DVE_EVACS:
                            nc.vector.tensor_copy(dv, sv)
                        else:
                            nc.scalar.copy(dv, sv)
                        ev[0] += 1
            pend.append((s, stage1(s, vrep0)))
            if len(pend) == 3:
                ps_, t1v_ = pend.pop(0)
                stage2(ps_, t1v_, pa, pb, True)
        while pend:
            ps_, t1v_ = pend.pop(0)
            stage2(ps_, t1v_, pa, pb, True)

        # ---------------- iter 1: v1, then pass 1 ----------------
        def sgview(t):
            return t.rearrange("p (n d) -> p n d", d=D)

        s1 = sml.tile([8, GJD], fp32, tag="s1")
        nc.scalar.copy(s1[:, 0:320], pa)
        nc.scalar.copy(s1[:, 320:640], pb)
        v1h = squash(s1, 8, G * J, "vv", f16, sgview)
        # vrep1 [128, (g, d, j)] from v1h [8, (g, j, d)]
        v1v = v1h.rearrange("p (g j d) -> p g d j", g=G, j=J)
        for half in range(2):
            pv = psV.tile([128, GJD // 2], fp32, tag="pv")
            nc.tensor.matmul(pv, s8, v1v[:, half * 2:(half + 1) * 2],
                             start=True, stop=True)
            nc.scalar.copy(vrep1[:, half * 320:(half + 1) * 320], pv)

        pa2 = psS.tile([8, GJD // 2], fp32, tag="pa")
        pb2 = psS.tile([8, GJD // 2], fp32, tag="pb")
        pend = []
        for s in range(NSLAB):
            pend.append((s, stage1(s, vrep1)))
            if len(pend) == 3:
                ps_, t1v_ = pend.pop(0)
                stage2(ps_, t1v_, pa2, pb2, False)
        while pend:
            ps_, t1v_ = pend.pop(0)
            stage2(ps_, t1v_, pa2, pb2, False)

        # ---------------- iter 2: v2 -> out ----------------
        s2 = sml.tile([8, GJD], fp32, tag="s2")
        nc.scalar.copy(s2[:, 0:320], pa2)
        nc.scalar.copy(s2[:, 320:640], pb2)
        v2 = squash(s2, 8, G * J, "vv", fp32, sgview)
        nc.sync.dma_start(out=t_out[:], in_=v2)

    nc.finalize()
    return nc


def _host_prep(x_full, W, bias):
    W = np.asarray(W, np.float32)
    wa = W.reshape(C, 16, J, E, D).transpose(1, 3, 0, 4, 2)  # [i16,e,c,d,j]
    wa = np.ascontiguousarray(wa.reshape(128, C * JD)).astype(F16)
    b2 = np.asarray(bias, np.float32).reshape(I, J)
    br = b2.reshape(C, 16, J).transpose(1, 0, 2)             # [i16,c,j]
    biasl = np.ascontiguousarray(
        np.broadcast_to(br[:, None], (16, 8, C, J)).reshape(128, C * J)
    ).astype(F16)
    m8x = np.zeros((128, 8, SN), F16)
    m8x[np.arange(128), np.arange(128) % 8, :] = 1
    m8x = m8x.reshape(128, 8 * SN)
    s8 = np.zeros((8, 128), F16)
    s8[np.arange(128) % 8, np.arange(128)] = 1
    s32 = np.zeros((32, 512), F16)
    for g in range(G):
        s32[g * 8 + np.arange(128) % 8, g * 128 + np.arange(128)] = 1
    maps = []
    for cl in range(NCORES):
        xl = np.asarray(x_full[cl * BL:(cl + 1) * BL], np.float32)
        inpT = xl.reshape(BL, C, 16, E).transpose(2, 3, 1, 0)  # [i16,e,c,b]
        inpT = np.ascontiguousarray(inpT.reshape(128, C * BL)).astype(F16)
        xr = xl.reshape(G, 8, C, 16, E)                        # [g,b8,c,i,e]
        z = np.zeros((16, 8, C, G, 16, 8), F16)
        for i in range(16):
            z[i, :, :, :, i, :] = xr[:, :, :, i, :].transpose(3, 2, 0, 1)
        ablk = z.reshape(128, C * G * 128)
        maps.append({"wa": wa, "inpT": inpT, "ablk": ablk, "biasl": biasl,
                     "m8x": m8x, "s8": s8, "s32": s32})
    return maps


_NC_CACHE = {}


def kernel(inputs, W, bias):
    from concourse import bass_utils

    if "nc" not in _NC_CACHE:
        _NC_CACHE["nc"] = _build_kernel()
    nc = _NC_CACHE["nc"]
    in_maps = _host_prep(inputs, W, bias)
    res = bass_utils.run_bass_kernel_spmd(nc, in_maps,
                                          core_ids=list(range(NCORES)))
    outs = []
    for r in res.results:
        v = r["out"].reshape(8, G, J, D).transpose(1, 0, 2, 3)  # [g,b8,j,d]
        outs.append(v.reshape(BL, J, D))
    return np.concatenate(outs, axis=0).astype(np.float32)


if __name__ == "__main__":
    import reference
    ins = reference.setup_inputs()
    ins = {k: np.asarray(v) for k, v in ins.items()}
    exp = np.asarray(reference.reference(**ins))
    got = kernel(**ins)
    err = np.abs(got - exp).max() / (np.abs(exp).max() + 1e-9)
    print("Relative error:", err)
